# revision 31
# baseline (speedup 1.0000x reference)
"""Trainium2 (8 NeuronCore) Bass kernel for the ActorNetwork GNN.

Self-contained: hardcodes the reference's static structure
(N=1048576 nodes, 8.4M edges, 2048 jobs x 512 ops, 64 envs x 32 jobs, 50 workers).

Host side does only integer index preprocessing (sorts / searchsorted / layout
permutation of inputs) and final unpermute/concat of outputs; all float math of
the network runs on the 8 NeuronCores.

Sharding: nodes in 8 contiguous ranges of 131072; edges dest(row)-sorted and
split at node boundaries. GCN aggregation:
  deg[v] = colcount(v)+1,  dinv = deg^-1/2,  g = dinv * mlp1(x)
  aggr[r] = dinv[r] * (sum_{e:row=r} g[col_e] + g[r])
g is AllGathered (32MB) so each core serves its own edges' gathers locally via
SWDGE indirect DMA; segment sums via prefix-scan over dest-sorted messages +
boundary-difference gathers (host-precomputed indptr grids).

SBUF layout: "pack4 quarter" — node v_local = 4*(FQ*q + f) + i maps to
partition p = q*4C + i*C + ch, free f (C = channels of that table).
"""

import sys
import numpy as np
from dataclasses import dataclass

sys.path.insert(0, "/opt/trn_rl_repo")

import concourse.bass as bass
import concourse.mybir as mybir
import concourse.tile as tile
from concourse import bacc
from concourse.bass_utils import run_bass_kernel_spmd

FP = mybir.dt.float32
I32 = mybir.dt.int32
AX = mybir.AxisListType
OP = mybir.AluOpType
AF = mybir.ActivationFunctionType


@dataclass(frozen=True)
class Cfg:
    NC: int = 8          # cores
    NPC: int = 131072    # nodes per core (16 * FQ)
    OPJ: int = 512       # ops per job
    JPE: int = 32        # jobs per env
    KCH: int = 256       # free size of one edge-chunk row (chunk = 128*KCH)
    NCHUNK: int = 33     # edge chunks per core (padded)
    NW: int = 50         # n_workers

    @property
    def N(self):
        return self.NC * self.NPC

    @property
    def E(self):
        return 8 * self.N

    @property
    def FQ(self):
        return self.NPC // 16

    @property
    def CH(self):
        return 128 * self.KCH

    @property
    def EPC(self):
        return self.NCHUNK * self.CH

    @property
    def JPC(self):
        return self.NPC // self.OPJ

    @property
    def EVC(self):
        return self.JPC // self.JPE

    @property
    def JQ(self):
        return self.FQ // (self.OPJ // 4)   # jobs per quarter

    @property
    def W1(self):
        return self.NW + 1

    @property
    def WPAD(self):
        return ((self.W1 + 3) // 4) * 4

    @property
    def MPRL(self):
        return self.JPC * self.WPAD // 4

    @property
    def NB(self):
        return self.NPC // 512              # boundary-grid blocks

    @property
    def CCQ(self):
        return self.NB // 4                 # 128-node blocks per quarter


FULL = Cfg()

# HW consumes indirect-DMA offset grids column-major (descriptor d reads
# offsets[d % 128, d // 128]); CoreSim consumes them row-major. Host lays
# out offsets accordingly.
HW_OFFSET_ORDER = False


def _hw_off(grid):
    """[128, M] desired row-major offset grid -> layout the HW consumes."""
    if not HW_OFFSET_ORDER:
        return grid
    m = grid.shape[1]
    return np.ascontiguousarray(grid.reshape(128 * m)[
        np.arange(128 * m).reshape(128, m, order="F")])


def _f32(a):
    return np.ascontiguousarray(a, dtype=np.float32)


def _i32(a):
    return np.ascontiguousarray(a, dtype=np.int32)


# ---------------------------------------------------------------------------
# Host preprocessing
# ---------------------------------------------------------------------------

def host_prep(cfg, x, edge_index, params):
    N, NPC, NC, FQ = cfg.N, cfg.NPC, cfg.NC, cfg.FQ

    row = np.asarray(edge_index[0], dtype=np.int64)
    col = np.asarray(edge_index[1], dtype=np.int64)
    x = np.asarray(x, dtype=np.float32)

    order = np.argsort(row, kind="stable")
    rs = row[order]
    cs = col[order].astype(np.int32)
    bounds = np.searchsorted(rs, np.arange(0, N + 1, NPC))

    csort = np.sort(col)
    ipcol = np.searchsorted(csort, np.arange(N + 1)).astype(np.int64)

    shared = {}
    per_core = []

    def pack_rows(arr_v, C):
        # arr_v [NPC, C] -> [16C, FQ], row p = q*4C + i*C + ch
        a = arr_v.reshape(4, FQ, 4, C)
        return a.transpose(0, 2, 3, 1).reshape(16 * C, FQ)

    def pack_rows_pad32(arr_v, C):
        # like pack_rows but each quarter block padded to 32 rows (PE base rule)
        a = arr_v.reshape(4, FQ, 4, C).transpose(0, 2, 3, 1)  # (q, i, ch, f)
        out = np.zeros((128, FQ), np.float32)
        for q in range(4):
            out[32 * q:32 * q + 4 * C] = a[q].reshape(4 * C, FQ)
        return out

    def expand8(arr16):
        # [16, FQ] -> [128, FQ] rows (q,i) replicated to (q,i,ch) for C=8
        return np.repeat(arr16, 8, axis=0)

    for c in range(NC):
        base = c * NPC
        e0, e1 = int(bounds[c]), int(bounds[c + 1])
        ec = e1 - e0
        assert ec <= cfg.EPC, f"core {c}: {ec} edges > EPC {cfg.EPC}"
        colpad = np.zeros(cfg.EPC, dtype=np.int32)
        colpad[:ec] = cs[e0:e1]
        cols = np.stack([_hw_off(ch.reshape(128, cfg.KCH))
                         for ch in colpad.reshape(cfg.NCHUNK, -1)])

        ip = np.searchsorted(rs[e0:e1], base + np.arange(NPC + 1)).astype(np.int32)

        def grid(a):
            gg = a.reshape(cfg.NB, 128, 4).transpose(1, 0, 2)
            gg = np.ascontiguousarray(gg.reshape(128, cfg.NB * 4))
            # per-quarter gather instructions each consume their own block
            CQ4 = cfg.CCQ * 4
            parts = [_hw_off(np.ascontiguousarray(gg[:, q*CQ4:(q+1)*CQ4]))
                     for q in range(4)]
            return np.concatenate(parts, axis=1)

        vids = base + np.arange(NPC, dtype=np.int64)
        ipa = expand8(pack_rows(ipcol[vids].reshape(NPC, 1), 1)).astype(np.int32)
        ipb = expand8(pack_rows(ipcol[vids + 1].reshape(NPC, 1), 1)).astype(np.int32)

        per_core.append({
            "xp": _f32(pack_rows_pad32(x[base:base + NPC], 5)),
            "cols": _i32(cols),
            "bstart": _i32(grid(ip[:-1])),
            "bend": _i32(grid(ip[1:])),
            "ipa": _i32(ipa),
            "ipb": _i32(ipb),
        })

    def W(p):
        return np.asarray(p, dtype=np.float32)

    def kron4(w):
        return _f32(np.kron(np.eye(4, dtype=np.float32), W(w)))

    def qrep(w):
        # [K, M] block-diag lhsT replicated into 4 base-32 quarter slots
        w = np.asarray(w, np.float32)
        out = np.zeros((128, w.shape[1]), np.float32)
        for q in range(4):
            out[32 * q:32 * q + w.shape[0]] = w
        return _f32(out)

    def tile4(w):
        return _f32(np.tile(W(w), (1, 4)))

    def bias4(b):
        return _f32(np.tile(W(b), 4).reshape(-1, 1))

    def bias1(b):
        return _f32(W(b).reshape(-1, 1))

    p1, p2 = params["mlp1"], params["mlp2"]
    pd, pg = params["mlp_dag"], params["mlp_global"]
    po, pp = params["mlp_op"], params["mlp_prlvl"]

    shared["w1a"], shared["w1b"], shared["w1c"] = (
        qrep(kron4(p1[0][0])), kron4(p1[1][0]), kron4(p1[2][0]))
    shared["b1a"], shared["b1b"], shared["b1c"] = (
        bias4(p1[0][1]), bias4(p1[1][1]), bias4(p1[2][1]))
    shared["w2a"], shared["w2b"], shared["w2c"] = (
        qrep(kron4(p2[0][0])), kron4(p2[1][0]), kron4(p2[2][0]))
    shared["b2a"], shared["b2b"], shared["b2c"] = (
        bias4(p2[0][1]), bias4(p2[1][1]), bias4(p2[2][1]))

    wd1 = W(pd[0][0])
    shared["wdx"], shared["wdxn"] = _f32(wd1[:5]), _f32(wd1[5:])
    shared["bd1"] = bias1(pd[0][1])
    shared["wd2"], shared["bd2"] = _f32(W(pd[1][0])), bias1(pd[1][1])
    shared["wd3"], shared["bd3"] = _f32(W(pd[2][0])), bias1(pd[2][1])
    shared["wg1"], shared["bg1"] = _f32(W(pg[0][0])), bias1(pg[0][1])
    shared["wg2"], shared["bg2"] = _f32(W(pg[1][0])), bias1(pg[1][1])
    shared["wg3"], shared["bg3"] = _f32(W(pg[2][0])), bias1(pg[2][1])

    wo1 = W(po[0][0])
    shared["wox"] = qrep(kron4(wo1[0:8]))
    shared["woy"] = tile4(wo1[8:16])
    shared["woz"] = tile4(wo1[16:24])
    shared["bo1"] = bias4(po[0][1])
    shared["wo2"], shared["bo2"] = kron4(po[1][0]), bias4(po[1][1])
    shared["wo3"], shared["bo3"] = kron4(po[2][0]), bias4(po[2][1])

    wp1 = W(pp[0][0])
    shared["wpl"] = kron4(wp1[0:1])
    shared["wpy"] = tile4(wp1[1:9])
    shared["wpz"] = tile4(wp1[9:17])
    shared["bp1"] = bias4(pp[0][1])
    shared["wp2"], shared["bp2"] = kron4(pp[1][0]), bias4(pp[1][1])
    shared["wp3"], shared["bp3"] = kron4(pp[2][0]), bias4(pp[2][1])

    r = np.arange(cfg.JPC * cfg.WPAD)
    shared["limits"] = _f32(
        (r % cfg.WPAD).astype(np.float32).reshape(cfg.MPRL, 4).T)

    # selx: [128, 20] — px rows live at 32q + i*5 + ch (padded quarters)
    selx = np.zeros((128, 20), np.float32)
    for q in range(4):
        for i in range(4):
            for ch in range(5):
                selx[32 * q + i * 5 + ch, 5 * q + ch] = 1.0
    shared["selx"] = _f32(selx)
    shared["selxn"] = _f32(np.kron(np.eye(4), np.kron(np.ones((4, 1)), np.eye(8))))
    shared["triu"] = _f32(np.triu(np.ones((128, 128)), k=1))
    shared["id128"] = _f32(np.eye(128))
    shared["id4"] = _f32(np.tile(np.eye(32, dtype=np.float32), (4, 1)))
    shared["ones1"] = _f32(np.ones((1, 128)))
    shared["onesc"] = _f32(np.ones((128, 1)))
    shared["one11"] = _f32(np.ones((1, 1)))

    return shared, per_core


# ---------------------------------------------------------------------------
# Bass builder
# ---------------------------------------------------------------------------

QNAMES = ["qPoolDynamic", "qPoolDynamic1", "qPoolDynamic2",
          "qPoolDynamic3"]


def build(cfg, shared_arrs, dbg=False):
    nc = bacc.Bacc("TRN2", target_bir_lowering=False, debug=False,
                   num_devices=cfg.NC)
    FQ, KCH, NCH = cfg.FQ, cfg.KCH, cfg.NCHUNK
    NF = min(512, FQ)
    NFC = FQ // NF
    RJ = cfg.OPJ // 4          # free elems per job per row
    CCQ = cfg.CCQ

    io = {}

    def param(name, shape, dtype=FP):
        io[name] = nc.declare_dram_parameter(name, list(shape), dtype,
                                             isOutput=False)
        return io[name]

    param("xp", (128, FQ))
    param("cols", (NCH, 128, KCH), I32)
    param("bstart", (128, cfg.NB * 4), I32)
    param("bend", (128, cfg.NB * 4), I32)
    param("ipa", (128, FQ), I32)
    param("ipb", (128, FQ), I32)
    for name, arr in shared_arrs.items():
        param(name, arr.shape)

    ops_out = nc.declare_dram_parameter("ops_out", [16, FQ], FP, isOutput=True)
    prl_out = nc.declare_dram_parameter("prl_out", [4, cfg.MPRL], FP,
                                        isOutput=True)
    dbg_t = {}
    if dbg:
        for nm in ["dbg_dinv", "dbg_g", "dbg_aggr"]:
            dbg_t[nm] = nc.declare_dram_parameter(nm, [128, FQ], FP,
                                                  isOutput=True)
        dbg_t["dbg_y"] = nc.declare_dram_parameter("dbg_y", [8, cfg.JPC], FP,
                                                   isOutput=True)
        dbg_t["dbg_msg"] = nc.declare_dram_parameter(
            "dbg_msg", [128, KCH, 8], FP, isOutput=True)
        dbg_t["dbg_raw"] = nc.declare_dram_parameter(
            "dbg_raw", [128, KCH, 8], FP, isOutput=True)
        dbg_t["dbg_raw2"] = nc.declare_dram_parameter(
            "dbg_raw2", [128, KCH, 8], FP, isOutput=True)

    replica = [list(range(cfg.NC))]

    with tile.TileContext(nc) as tc:
        with (
            tc.tile_pool(name="dram", bufs=1, space="DRAM") as dpool,
            tc.tile_pool(name="const", bufs=1) as cpool,
            tc.tile_pool(name="big", bufs=1) as bigpool,
            tc.tile_pool(name="work", bufs=2) as wpool,
            tc.tile_pool(name="small", bufs=1) as spool,
            tc.tile_pool(name="msg", bufs=2) as mpool,
            tc.tile_pool(name="psA", bufs=2, space="PSUM") as psA,
            tc.tile_pool(name="psB", bufs=2, space="PSUM") as psB,
            tc.tile_pool(name="psC", bufs=2, space="PSUM") as psC,
            tc.tile_pool(name="psS", bufs=2, space="PSUM") as psS,
        ):
            g_slice = dpool.tile([cfg.NPC, 8], FP, name="g_slice")
            g_full = dpool.tile([cfg.N, 8], FP, name="g_full",
                                addr_space="Shared")
            cum = dpool.tile([cfg.EPC + 1, 8], FP, name="cum")

            sb = {}
            for name, arr in shared_arrs.items():
                if name == "limits":
                    continue
                t = cpool.tile(list(arr.shape), FP, name=f"c_{name}")
                nc.sync.dma_start(out=t[:], in_=io[name][:])
                sb[name] = t

            # ---- Phase 1: dinv [128, FQ] (tag T3) ----
            ipa_s = bigpool.tile([128, FQ], I32, name="ipa_s", tag="T1")
            ipb_s = bigpool.tile([128, FQ], I32, name="ipb_s", tag="T2")
            nc.sync.dma_start(out=ipa_s[:], in_=io["ipa"][:])
            nc.sync.dma_start(out=ipb_s[:], in_=io["ipb"][:])
            nc.vector.tensor_tensor(out=ipb_s[:], in0=ipb_s[:], in1=ipa_s[:],
                                    op=OP.subtract)
            dinv = bigpool.tile([128, FQ], FP, name="dinv", tag="T3")
            nc.vector.tensor_copy(out=dinv[:], in_=ipb_s[:])
            nc.vector.tensor_scalar(out=dinv[:], in0=dinv[:], scalar1=1.0,
                                    scalar2=None, op0=OP.add)
            nc.scalar.sqrt(out=dinv[:], in_=dinv[:])
            nc.vector.reciprocal(out=dinv[:], in_=dinv[:])
            if dbg:
                nc.sync.dma_start(out=dbg_t["dbg_dinv"][:], in_=dinv[:])

            # ---- Phase 2: mlp1 -> h (tag T1); g = dinv*h in place ----
            xp_s = bigpool.tile([128, FQ], FP, name="xp_s", tag="T4")
            nc.sync.dma_start(out=xp_s[:], in_=io["xp"][:])
            h = bigpool.tile([128, FQ], FP, name="h", tag="T1")

            def mlp3(rhs0, wa_ap, ba, wb, bb, wc, bc, m3, nf, tp=(0, 0)):
                ps1 = psA.tile([128, nf], FP, name="ps1", tag="A")
                nc.tensor.matmul(out=ps1[:], lhsT=wa_ap, rhs=rhs0,
                                 start=True, stop=True, tile_position=tp)
                a1 = wpool.tile([128, nf], FP, name="a1", tag="a1")
                nc.scalar.activation(out=a1[:], in_=ps1[:], func=AF.Relu,
                                     bias=sb[ba][:, 0:1])
                ps2 = psB.tile([64, nf], FP, name="ps2", tag="B")
                nc.tensor.matmul(out=ps2[:], lhsT=sb[wb][:], rhs=a1[:],
                                 start=True, stop=True)
                a2 = wpool.tile([64, nf], FP, name="a2", tag="a2")
                nc.scalar.activation(out=a2[:], in_=ps2[:], func=AF.Relu,
                                     bias=sb[bb][:, 0:1])
                ps3 = psC.tile([m3, nf], FP, name="ps3", tag="C")
                nc.tensor.matmul(out=ps3[:], lhsT=sb[wc][:], rhs=a2[:],
                                 start=True, stop=True)
                outc = wpool.tile([m3, nf], FP, name="outc", tag="t2k")
                nc.scalar.activation(out=outc[:], in_=ps3[:], func=AF.Identity,
                                     bias=sb[bc][:, 0:1])
                return outc

            for q in range(4):
                for f in range(NFC):
                    fs = slice(f * NF, (f + 1) * NF)
                    hc = mlp3(xp_s[32 * q:32 * q + 20, fs],
                              sb["w1a"][32 * q:32 * q + 20, :],
                              "b1a", "w1b", "b1b", "w1c", "b1c", 32, NF,
                              tp=(32 * q, 0))
                    nc.sync.dma_start(out=h[32 * q:32 * q + 32, fs], in_=hc[:])
            # px (x pooling) before xp slot is reused in phase 5
            px = spool.tile([128, cfg.JQ], FP, name="px")
            nc.vector.tensor_reduce(
                out=px[:], in_=xp_s[:].rearrange("p (j r) -> p j r", r=RJ),
                axis=AX.X, op=OP.add)
            # g = h * dinv (in place; tile "h" now holds g)
            nc.vector.tensor_tensor(out=h[:], in0=h[:], in1=dinv[:], op=OP.mult)
            g = h
            if dbg:
                nc.sync.dma_start(out=dbg_t["dbg_g"][:], in_=g[:])

            # ---- Phase 3: g table -> DRAM node-major + AllGather ----
            TB = min(8, FQ // 128)
            for q in range(4):
                for cb in range(FQ // (128 * TB)):
                    pst = psC.tile([128, 32 * TB], FP, name="pst", tag="C")
                    for t in range(TB):
                        f0 = cb * 128 * TB + t * 128
                        nc.tensor.transpose(
                            out=pst[:, 32 * t:32 * t + 32],
                            in_=g[32 * q:32 * q + 32, f0:f0 + 128],
                            identity=sb["id4"][32 * q:32 * q + 32, :],
                            tile_position=(32 * q, 0))
                    gt = wpool.tile([128, 32 * TB], FP, name="gt",
                                    tag="t2k")
                    nc.vector.tensor_copy(out=gt[:], in_=pst[:])
                    node0 = 4 * (FQ * q + cb * 128 * TB)
                    nc.sync.dma_start(
                        out=g_slice[node0:node0 + 512 * TB, :].rearrange(
                            "(t j i) d -> j t i d", t=TB, j=128, i=4),
                        in_=gt[:])
            nc.gpsimd.collective_compute(
                "AllGather", OP.bypass, replica_groups=replica,
                ins=[g_slice[:]], outs=[g_full[:]])

            # ---- Phase 4: gather + scan + cum ----
            zrow = spool.tile([1, 8], FP, name="zrow")
            nc.vector.memset(zrow[:], 0.0)
            nc.sync.dma_start(out=cum[0:1, :], in_=zrow[:])
            gc0 = spool.tile([1, 8], FP, name="gc0")
            nc.vector.memset(gc0[:], 0.0)
            for c in range(NCH):
                colt = mpool.tile([128, KCH], I32, name="colt", tag="colt")
                nc.sync.dma_start(out=colt[:], in_=io["cols"][c])
                msg = mpool.tile([128, KCH, 8], FP, name="msg", tag="msg")
                for jj in range(KCH):
                    nc.gpsimd.indirect_dma_start(
                        out=msg[:, jj, :], out_offset=None, in_=g_full[:],
                        in_offset=bass.IndirectOffsetOnAxis(
                            ap=colt[:, jj:jj + 1], axis=0))
                if dbg and c == NCH - 1:
                    nc.sync.dma_start(out=dbg_t["dbg_raw"][:], in_=msg[:])
                    last_msg = msg
                sc = mpool.tile([128, KCH, 8], FP, name="sc", tag="scan",
                                bufs=1)
                for chn in range(8):
                    nc.vector.tensor_tensor_scan(
                        out=sc[:, :, chn], data0=msg[:, :, chn],
                        data1=msg[:, :, chn], initial=0.0,
                        op0=OP.add, op1=OP.bypass)
                tots = wpool.tile([128, 8], FP, name="tots", tag="tots")
                nc.vector.tensor_copy(out=tots[:], in_=sc[:, KCH - 1, :])
                cps = psS.tile([128, 8], FP, name="cps", tag="S")
                nc.tensor.matmul(out=cps[:], lhsT=sb["triu"][:], rhs=tots[:],
                                 start=True, stop=False)
                nc.tensor.matmul(out=cps[:], lhsT=sb["ones1"][:],
                                 rhs=gc0[:], start=False, stop=True)
                ca = wpool.tile([128, 8], FP, name="ca", tag="ca")
                nc.vector.tensor_copy(out=ca[:], in_=cps[:])
                gps = psS.tile([1, 8], FP, name="gps", tag="S")
                nc.tensor.matmul(out=gps[:], lhsT=sb["onesc"][:], rhs=tots[:],
                                 start=True, stop=False)
                nc.tensor.matmul(out=gps[:], lhsT=sb["one11"][:], rhs=gc0[:],
                                 start=False, stop=True)
                nc.vector.tensor_copy(out=gc0[:], in_=gps[:])
                nc.vector.tensor_tensor(
                    out=sc[:], in0=sc[:],
                    in1=ca[:].unsqueeze(1).broadcast_to([128, KCH, 8]),
                    op=OP.add)
                nc.sync.dma_start(
                    out=cum[1 + c * cfg.CH: 1 + (c + 1) * cfg.CH, :].rearrange(
                        "(p k) d -> p (k d)", p=128),
                    in_=sc[:].rearrange("p k d -> p (k d)"))
                if dbg and c == NCH - 1:
                    nc.sync.dma_start(out=dbg_t["dbg_msg"][:], in_=sc[:])

            # ---- Phase 5: boundary gathers + diff -> aggr (pack4) ----
            aggr = bigpool.tile([128, FQ], FP, name="aggr", tag="T2")
            TB2 = min(4, CCQ)
            for q in range(4):
                qs = slice(q * CCQ * 4, (q + 1) * CCQ * 4)
                bs_q = mpool.tile([128, CCQ * 4], I32, name="bs_q", tag="colt")
                be_q = mpool.tile([128, CCQ * 4], I32, name="be_q", tag="colt")
                nc.sync.dma_start(out=bs_q[:], in_=io["bstart"][:, qs])
                nc.sync.dma_start(out=be_q[:], in_=io["bend"][:, qs])
                stq = bigpool.tile([128, CCQ * 4, 8], FP, name="stq", tag="T4")
                enq = bigpool.tile([128, CCQ * 4, 8], FP, name="enq", tag="enq")
                for jj in range(CCQ * 4):
                    nc.gpsimd.indirect_dma_start(
                        out=stq[:, jj, :], out_offset=None, in_=cum[:],
                        in_offset=bass.IndirectOffsetOnAxis(
                            ap=bs_q[:, jj:jj + 1], axis=0))
                    nc.gpsimd.indirect_dma_start(
                        out=enq[:, jj, :], out_offset=None, in_=cum[:],
                        in_offset=bass.IndirectOffsetOnAxis(
                            ap=be_q[:, jj:jj + 1], axis=0))
                nc.vector.tensor_tensor(
                    out=enq[:].rearrange("p b d -> p (b d)"),
                    in0=enq[:].rearrange("p b d -> p (b d)"),
                    in1=stq[:].rearrange("p b d -> p (b d)"), op=OP.subtract)
                for cb in range(CCQ // TB2):
                    pst = psC.tile([32, 128 * TB2], FP, name="psa", tag="C")
                    for t in range(TB2):
                        lc = cb * TB2 + t
                        nc.tensor.transpose(
                            out=pst[:, 128 * t:128 * t + 128],
                            in_=enq[:, 4 * lc:4 * lc + 4, :].rearrange(
                                "p i d -> p (i d)"),
                            identity=sb["id128"][:])
                    at = wpool.tile([32, 128 * TB2], FP, name="at",
                                    tag="t2k")
                    nc.vector.tensor_copy(out=at[:], in_=pst[:])
                    f0 = cb * TB2 * 128
                    nc.sync.dma_start(
                        out=aggr[32 * q:32 * q + 32, f0:f0 + 128 * TB2],
                        in_=at[:])

            nc.vector.tensor_tensor(out=aggr[:], in0=aggr[:], in1=g[:],
                                    op=OP.add)
            nc.vector.tensor_tensor(out=aggr[:], in0=aggr[:], in1=dinv[:],
                                    op=OP.mult)
            if dbg:
                nc.sync.dma_start(out=dbg_t["dbg_aggr"][:], in_=aggr[:])

            # ---- Phase 6: mlp2 -> x_node (tag T1, reuses g/h slot) ----
            xn = bigpool.tile([128, FQ], FP, name="xn", tag="T1")
            for q in range(4):
                for f in range(NFC):
                    fs = slice(f * NF, (f + 1) * NF)
                    xc = mlp3(aggr[32 * q:32 * q + 32, fs],
                              sb["w2a"][32 * q:32 * q + 32, :],
                              "b2a", "w2b", "b2b", "w2c", "b2c", 32, NF,
                              tp=(32 * q, 0))
                    nc.sync.dma_start(out=xn[32 * q:32 * q + 32, fs], in_=xc[:])

            # ---- Phase 7: pooling + mlp_dag + mlp_global ----
            pxn = spool.tile([128, cfg.JQ], FP, name="pxn")
            nc.vector.tensor_reduce(
                out=pxn[:], in_=xn[:].rearrange("p (j r) -> p j r", r=RJ),
                axis=AX.X, op=OP.add)
            sxp = psS.tile([5, cfg.JPC], FP, name="sxp", tag="S")
            sxnp = psS.tile([8, cfg.JPC], FP, name="sxnp", tag="S")
            for q in range(4):
                js = slice(q * cfg.JQ, (q + 1) * cfg.JQ)
                nc.tensor.matmul(out=sxp[:, js],
                                 lhsT=sb["selx"][:, 5 * q:5 * q + 5],
                                 rhs=px[:], start=True, stop=True)
                nc.tensor.matmul(out=sxnp[:, js],
                                 lhsT=sb["selxn"][:, 8 * q:8 * q + 8],
                                 rhs=pxn[:], start=True, stop=True)
            sx = spool.tile([5, cfg.JPC], FP, name="sx_s")
            sxn = spool.tile([8, cfg.JPC], FP, name="sxn_s")
            nc.vector.tensor_copy(out=sx[:], in_=sxp[:])
            nc.vector.tensor_copy(out=sxn[:], in_=sxnp[:])

            def mlp3s(rhs_list, ws, bs_, ms, name):
                cols_n = rhs_list[0][1].shape[-1]
                ps = psS.tile([ms[0], cols_n], FP, name=f"{name}p1", tag="S")
                for j, (wn, rap) in enumerate(rhs_list):
                    nc.tensor.matmul(out=ps[:], lhsT=sb[wn][:], rhs=rap,
                                     start=(j == 0),
                                     stop=(j == len(rhs_list) - 1))
                o1 = spool.tile([ms[0], cols_n], FP, name=f"{name}o1")
                nc.scalar.activation(out=o1[:], in_=ps[:], func=AF.Relu,
                                     bias=sb[bs_[0]][:, 0:1])
                ps2 = psS.tile([ms[1], cols_n], FP, name=f"{name}p2", tag="S")
                nc.tensor.matmul(out=ps2[:], lhsT=sb[ws[1]][:], rhs=o1[:],
                                 start=True, stop=True)
                o2 = spool.tile([ms[1], cols_n], FP, name=f"{name}o2")
                nc.scalar.activation(out=o2[:], in_=ps2[:], func=AF.Relu,
                                     bias=sb[bs_[1]][:, 0:1])
                ps3 = psS.tile([ms[2], cols_n], FP, name=f"{name}p3", tag="S")
                nc.tensor.matmul(out=ps3[:], lhsT=sb[ws[2]][:], rhs=o2[:],
                                 start=True, stop=True)
                o3 = spool.tile([ms[2], cols_n], FP, name=f"{name}o3")
                nc.scalar.activation(out=o3[:], in_=ps3[:], func=AF.Identity,
                                     bias=sb[bs_[2]][:, 0:1])
                return o3

            y = mlp3s([("wdx", sx[:]), ("wdxn", sxn[:])],
                      [None, "wd2", "wd3"], ["bd1", "bd2", "bd3"],
                      [32, 16, 8], "dag")
            if dbg:
                nc.sync.dma_start(out=dbg_t["dbg_y"][:], in_=y[:])
                nc.sync.dma_start(out=dbg_t["dbg_raw2"][:], in_=last_msg[:])
            yz = spool.tile([8, cfg.EVC], FP, name="yz")
            nc.vector.tensor_reduce(
                out=yz[:], in_=y[:].rearrange("p (e j) -> p e j", j=cfg.JPE),
                axis=AX.X, op=OP.add)
            z = mlp3s([("wg1", yz[:])],
                      [None, "wg2", "wg3"], ["bg1", "bg2", "bg3"],
                      [32, 16, 8], "glb")

            # ---- Phase 8: mlp_op -> ops_out ----
            JC = NF // RJ               # jobs per free-chunk
            EPN = cfg.OPJ * cfg.JPE // 4  # free elems per env per row
            for q in range(4):
                for f in range(NFC):
                    fs = slice(f * NF, (f + 1) * NF)
                    j0 = q * cfg.JQ + f * JC
                    e0 = (q * FQ + f * NF) // EPN
                    yr = wpool.tile([8, NF], FP, name="yr", tag="yr")
                    nc.vector.tensor_copy(
                        out=yr[:].rearrange("p (j r) -> p j r", r=RJ),
                        in_=y[:, j0:j0 + JC].unsqueeze(2).broadcast_to(
                            [8, JC, RJ]))
                    zr = wpool.tile([8, NF], FP, name="zr", tag="zr")
                    if EPN >= NF:
                        nc.vector.tensor_copy(
                            out=zr[:],
                            in_=z[:, e0:e0 + 1].broadcast_to([8, NF]))
                    else:
                        ec_n = NF // EPN
                        nc.vector.tensor_copy(
                            out=zr[:].rearrange("p (e r) -> p e r", r=EPN),
                            in_=z[:, e0:e0 + ec_n].unsqueeze(2).broadcast_to(
                                [8, ec_n, EPN]))
                    ps1 = psA.tile([128, NF], FP, name="po1", tag="A")
                    nc.tensor.matmul(out=ps1[:],
                                     lhsT=sb["wox"][32 * q:32 * q + 32, :],
                                     rhs=xn[32 * q:32 * q + 32, fs],
                                     start=True, stop=False,
                                     tile_position=(32 * q, 0))
                    nc.tensor.matmul(out=ps1[:], lhsT=sb["woy"][:], rhs=yr[:],
                                     start=False, stop=False)
                    nc.tensor.matmul(out=ps1[:], lhsT=sb["woz"][:], rhs=zr[:],
                                     start=False, stop=True)
                    a1 = wpool.tile([128, NF], FP, name="oa1", tag="a1")
                    nc.scalar.activation(out=a1[:], in_=ps1[:], func=AF.Relu,
                                         bias=sb["bo1"][:, 0:1])
                    ps2 = psB.tile([64, NF], FP, name="po2", tag="B")
                    nc.tensor.matmul(out=ps2[:], lhsT=sb["wo2"][:], rhs=a1[:],
                                     start=True, stop=True)
                    a2 = wpool.tile([64, NF], FP, name="oa2", tag="a2")
                    nc.scalar.activation(out=a2[:], in_=ps2[:], func=AF.Relu,
                                         bias=sb["bo2"][:, 0:1])
                    ps3 = psC.tile([4, NF], FP, name="po3", tag="C")
                    nc.tensor.matmul(out=ps3[:], lhsT=sb["wo3"][:], rhs=a2[:],
                                     start=True, stop=True)
                    oc = wpool.tile([4, NF], FP, name="oc", tag="t2k")
                    nc.scalar.activation(out=oc[:], in_=ps3[:],
                                         func=AF.Identity,
                                         bias=sb["bo3"][:, 0:1])
                    nc.sync.dma_start(out=ops_out[4 * q:4 * q + 4, fs],
                                      in_=oc[:])

            # ---- Phase 9: prlvl -> prl_out ----
            RW = cfg.WPAD // 4
            REW = cfg.WPAD * cfg.JPE // 4
            lim_s = bigpool.tile([4, cfg.MPRL], FP, name="lim_s", tag="T4")
            nc.sync.dma_start(out=lim_s[:], in_=io["limits"][:])
            yrp = bigpool.tile([8, cfg.MPRL], FP, name="yrp", tag="T3")
            nc.vector.tensor_copy(
                out=yrp[:].rearrange("p (j r) -> p j r", r=RW),
                in_=y[:].unsqueeze(2).broadcast_to([8, cfg.JPC, RW]))
            zrp = bigpool.tile([8, cfg.MPRL], FP, name="zrp", tag="enq")
            nc.vector.tensor_copy(
                out=zrp[:].rearrange("p (e r) -> p e r", r=REW),
                in_=z[:].unsqueeze(2).broadcast_to([8, cfg.EVC, REW]))
            f0 = 0
            while f0 < cfg.MPRL:
                nf = min(512, cfg.MPRL - f0)
                fs = slice(f0, f0 + nf)
                ps1 = psA.tile([128, nf], FP, name="pp1", tag="A")
                nc.tensor.matmul(out=ps1[:], lhsT=sb["wpl"][:],
                                 rhs=lim_s[:, fs], start=True, stop=False)
                nc.tensor.matmul(out=ps1[:], lhsT=sb["wpy"][:], rhs=yrp[:, fs],
                                 start=False, stop=False)
                nc.tensor.matmul(out=ps1[:], lhsT=sb["wpz"][:], rhs=zrp[:, fs],
                                 start=False, stop=True)
                a1 = wpool.tile([128, nf], FP, name="pa1", tag="a1")
                nc.scalar.activation(out=a1[:], in_=ps1[:], func=AF.Relu,
                                     bias=sb["bp1"][:, 0:1])
                ps2 = psB.tile([64, nf], FP, name="pp2", tag="B")
                nc.tensor.matmul(out=ps2[:], lhsT=sb["wp2"][:], rhs=a1[:],
                                 start=True, stop=True)
                a2 = wpool.tile([64, nf], FP, name="pa2", tag="a2")
                nc.scalar.activation(out=a2[:], in_=ps2[:], func=AF.Relu,
                                     bias=sb["bp2"][:, 0:1])
                ps3 = psC.tile([4, nf], FP, name="pp3", tag="C")
                nc.tensor.matmul(out=ps3[:], lhsT=sb["wp3"][:], rhs=a2[:],
                                 start=True, stop=True)
                oc = wpool.tile([4, nf], FP, name="poc", tag="t2k")
                nc.scalar.activation(out=oc[:], in_=ps3[:], func=AF.Identity,
                                     bias=sb["bp3"][:, 0:1])
                nc.sync.dma_start(out=prl_out[:, fs], in_=oc[:])
                f0 += nf

    nc.compile()
    return nc


# ---------------------------------------------------------------------------
# Host postprocessing
# ---------------------------------------------------------------------------

def host_post(cfg, results, batch, num_jobs_per_env):
    ops = []
    prl = []
    for c in range(cfg.NC):
        b = np.asarray(results[c]["ops_out"])          # [16, FQ]
        t = b.reshape(4, 4, cfg.FQ).transpose(0, 2, 1).reshape(cfg.NPC)
        ops.append(t)
        p = np.asarray(results[c]["prl_out"])          # [4, MPRL]
        rows = p.T.reshape(cfg.JPC * cfg.WPAD)
        prl.append(rows.reshape(cfg.JPC, cfg.WPAD)[:, :cfg.W1])
    op_scores = np.concatenate(ops).astype(np.float32)
    prlvl = np.concatenate(prl, axis=0).astype(np.float32)

    njpe = np.asarray(num_jobs_per_env)
    total_jobs = int(njpe.sum())
    batch = np.asarray(batch)
    nopj = np.bincount(batch, minlength=total_jobs).astype(np.int32)
    env_ids = np.repeat(np.arange(njpe.shape[0]), njpe)
    nope = np.zeros(njpe.shape[0], dtype=np.int32)
    np.add.at(nope, env_ids, nopj)
    job_indptr = np.concatenate([np.zeros(1, np.int32),
                                 np.cumsum(njpe).astype(np.int32)])
    return op_scores, prlvl, nope, job_indptr


# ---------------------------------------------------------------------------
# Entry points
# ---------------------------------------------------------------------------

_CACHE = {}


def get_nc(cfg, shared):
    if cfg not in _CACHE:
        _CACHE[cfg] = build(cfg, shared)
    return _CACHE[cfg]


def make_in_maps(cfg, shared, per_core):
    in_maps = []
    for c in range(cfg.NC):
        m = dict(shared)
        m.update(per_core[c])
        in_maps.append(m)
    return in_maps


def run(cfg, x, edge_index, batch, num_jobs_per_env, n_workers, params,
        trace=False):
    shared, per_core = host_prep(cfg, x, edge_index, params)
    nc = get_nc(cfg, shared)
    in_maps = make_in_maps(cfg, shared, per_core)
    res = run_bass_kernel_spmd(nc, in_maps, core_ids=list(range(cfg.NC)),
                               trace=trace)
    out = host_post(cfg, res.results, batch, num_jobs_per_env)
    return out, res


def kernel(x, edge_index, batch, num_jobs_per_env, n_workers, params):
    cfg = FULL
    assert int(n_workers) == cfg.NW
    out, _ = run(cfg, x, edge_index, batch, num_jobs_per_env, n_workers,
                 params)
    return out


# revision 34
# speedup vs baseline: 2.5080x; 2.5080x over previous
"""Trainium2 (8 NeuronCore) Bass kernel for the ActorNetwork GNN.

Self-contained: hardcodes the reference's static structure
(N=1048576 nodes, 8.4M edges, 2048 jobs x 512 ops, 64 envs x 32 jobs, 50 workers).

Host side does only integer index preprocessing (sorts / searchsorted / layout
permutation of inputs) and final unpermute/concat of outputs; all float math of
the network runs on the 8 NeuronCores.

Sharding: nodes in 8 contiguous ranges of 131072; edges dest(row)-sorted and
split at node boundaries. GCN aggregation:
  deg[v] = colcount(v)+1,  dinv = deg^-1/2,  g = dinv * mlp1(x)
  aggr[r] = dinv[r] * (sum_{e:row=r} g[col_e] + g[r])
g is AllGathered (32MB) so each core serves its own edges' gathers locally via
SWDGE indirect DMA; segment sums via prefix-scan over dest-sorted messages +
boundary-difference gathers (host-precomputed indptr grids).

SBUF layout: "pack4 quarter" — node v_local = 4*(FQ*q + f) + i maps to
partition p = q*4C + i*C + ch, free f (C = channels of that table).
"""

import sys
import numpy as np
from dataclasses import dataclass

sys.path.insert(0, "/opt/trn_rl_repo")

import concourse.bass as bass
import concourse.mybir as mybir
import concourse.tile as tile
from concourse import bacc
from concourse.bass_utils import run_bass_kernel_spmd

FP = mybir.dt.float32
I32 = mybir.dt.int32
AX = mybir.AxisListType
OP = mybir.AluOpType
AF = mybir.ActivationFunctionType


@dataclass(frozen=True)
class Cfg:
    NC: int = 8          # cores
    NPC: int = 131072    # nodes per core (16 * FQ)
    OPJ: int = 512       # ops per job
    JPE: int = 32        # jobs per env
    KCH: int = 256       # free size of one edge-chunk row (chunk = 128*KCH)
    NCHUNK: int = 33     # edge chunks per core (padded)
    NW: int = 50         # n_workers

    @property
    def N(self):
        return self.NC * self.NPC

    @property
    def E(self):
        return 8 * self.N

    @property
    def FQ(self):
        return self.NPC // 16

    @property
    def CH(self):
        return 128 * self.KCH

    @property
    def EPC(self):
        return self.NCHUNK * self.CH

    @property
    def JPC(self):
        return self.NPC // self.OPJ

    @property
    def EVC(self):
        return self.JPC // self.JPE

    @property
    def JQ(self):
        return self.FQ // (self.OPJ // 4)   # jobs per quarter

    @property
    def W1(self):
        return self.NW + 1

    @property
    def WPAD(self):
        return ((self.W1 + 3) // 4) * 4

    @property
    def MPRL(self):
        return self.JPC * self.WPAD // 4

    @property
    def NB(self):
        return self.NPC // 512              # boundary-grid blocks

    @property
    def CCQ(self):
        return self.NB // 4                 # 128-node blocks per quarter

    @property
    def FQE(self):
        return self.EPC // 16               # pack4 width of the edge stream

    @property
    def FSE(self):
        return min(2048, self.FQE)          # per-section f width (edge mlp)


FULL = Cfg()

# HW consumes indirect-DMA offset grids column-major (descriptor d reads
# offsets[d % 128, d // 128]); CoreSim consumes them row-major. Host lays
# out offsets accordingly.
HW_OFFSET_ORDER = False


def _hw_off(grid):
    """[128, M] desired row-major offset grid -> layout the HW consumes."""
    if not HW_OFFSET_ORDER:
        return grid
    m = grid.shape[1]
    return np.ascontiguousarray(grid.reshape(128 * m)[
        np.arange(128 * m).reshape(128, m, order="F")])


def _f32(a):
    return np.ascontiguousarray(a, dtype=np.float32)


def _i32(a):
    return np.ascontiguousarray(a, dtype=np.int32)


# ---------------------------------------------------------------------------
# Host preprocessing
# ---------------------------------------------------------------------------

def host_prep(cfg, x, edge_index, params):
    N, NPC, NC, FQ = cfg.N, cfg.NPC, cfg.NC, cfg.FQ

    row = np.asarray(edge_index[0], dtype=np.int64)
    col = np.asarray(edge_index[1], dtype=np.int64)
    x = np.asarray(x, dtype=np.float32)

    order = np.argsort(row, kind="stable")
    rs = row[order]
    cs = col[order].astype(np.int32)
    bounds = np.searchsorted(rs, np.arange(0, N + 1, NPC))

    csort = np.sort(col)
    ipcol = np.searchsorted(csort, np.arange(N + 1)).astype(np.int64)

    shared = {}
    per_core = []

    def pack_rows(arr_v, C, W=FQ):
        # arr_v [16W, C] -> [16C, W], row p = q*4C + i*C + ch
        a = arr_v.reshape(4, W, 4, C)
        return a.transpose(0, 2, 3, 1).reshape(16 * C, W)

    def pack_rows_pad32(arr_v, C, W=FQ):
        # like pack_rows but each quarter block padded to 32 rows (PE base rule)
        a = arr_v.reshape(4, W, 4, C).transpose(0, 2, 3, 1)  # (q, i, ch, f)
        out = np.zeros((128, W), np.float32)
        for q in range(4):
            out[32 * q:32 * q + 4 * C] = a[q].reshape(4 * C, W)
        return out

    def expand8(arr16):
        # [16, FQ] -> [128, FQ] rows (q,i) replicated to (q,i,ch) for C=8
        return np.repeat(arr16, 8, axis=0)

    for c in range(NC):
        base = c * NPC
        e0, e1 = int(bounds[c]), int(bounds[c + 1])
        ec = e1 - e0
        assert ec <= cfg.EPC, f"core {c}: {ec} edges > EPC {cfg.EPC}"
        colpad = np.zeros(cfg.EPC, dtype=np.int64)
        colpad[:ec] = cs[e0:e1]
        # per-edge streams in pack4(FQE) layout: x[col_e], ipcol[col_e(+1)]
        xe = pack_rows_pad32(x[colpad], 5, cfg.FQE)
        epa = np.repeat(pack_rows(
            ipcol[colpad].reshape(cfg.EPC, 1), 1, cfg.FQE), 8, axis=0)
        epb = np.repeat(pack_rows(
            ipcol[colpad + 1].reshape(cfg.EPC, 1), 1, cfg.FQE), 8, axis=0)

        ip = np.searchsorted(rs[e0:e1], base + np.arange(NPC + 1)).astype(np.int32)

        def grid(a):
            gg = a.reshape(cfg.NB, 128, 4).transpose(1, 0, 2)
            gg = np.ascontiguousarray(gg.reshape(128, cfg.NB * 4))
            # per-quarter gather instructions each consume their own block
            CQ4 = cfg.CCQ * 4
            parts = [_hw_off(np.ascontiguousarray(gg[:, q*CQ4:(q+1)*CQ4]))
                     for q in range(4)]
            return np.concatenate(parts, axis=1)

        vids = base + np.arange(NPC, dtype=np.int64)
        ipa = expand8(pack_rows(ipcol[vids].reshape(NPC, 1), 1)).astype(np.int32)
        ipb = expand8(pack_rows(ipcol[vids + 1].reshape(NPC, 1), 1)).astype(np.int32)

        per_core.append({
            "xp": _f32(pack_rows_pad32(x[base:base + NPC], 5)),
            "xe": _f32(xe),
            "epa": _i32(epa),
            "epb": _i32(epb),
            "bstart": _i32(grid(ip[:-1])),
            "bend": _i32(grid(ip[1:])),
            "ipa": _i32(ipa),
            "ipb": _i32(ipb),
        })

    def W(p):
        return np.asarray(p, dtype=np.float32)

    def kron4(w):
        return _f32(np.kron(np.eye(4, dtype=np.float32), W(w)))

    def qrep(w):
        # [K, M] block-diag lhsT replicated into 4 base-32 quarter slots
        w = np.asarray(w, np.float32)
        out = np.zeros((128, w.shape[1]), np.float32)
        for q in range(4):
            out[32 * q:32 * q + w.shape[0]] = w
        return _f32(out)

    def tile4(w):
        return _f32(np.tile(W(w), (1, 4)))

    def bias4(b):
        return _f32(np.tile(W(b), 4).reshape(-1, 1))

    def bias1(b):
        return _f32(W(b).reshape(-1, 1))

    p1, p2 = params["mlp1"], params["mlp2"]
    pd, pg = params["mlp_dag"], params["mlp_global"]
    po, pp = params["mlp_op"], params["mlp_prlvl"]

    shared["w1a"], shared["w1b"], shared["w1c"] = (
        qrep(kron4(p1[0][0])), kron4(p1[1][0]), kron4(p1[2][0]))
    shared["b1a"], shared["b1b"], shared["b1c"] = (
        bias4(p1[0][1]), bias4(p1[1][1]), bias4(p1[2][1]))
    shared["w2a"], shared["w2b"], shared["w2c"] = (
        qrep(kron4(p2[0][0])), kron4(p2[1][0]), kron4(p2[2][0]))
    shared["b2a"], shared["b2b"], shared["b2c"] = (
        bias4(p2[0][1]), bias4(p2[1][1]), bias4(p2[2][1]))

    wd1 = W(pd[0][0])
    shared["wdx"], shared["wdxn"] = _f32(wd1[:5]), _f32(wd1[5:])
    shared["bd1"] = bias1(pd[0][1])
    shared["wd2"], shared["bd2"] = _f32(W(pd[1][0])), bias1(pd[1][1])
    shared["wd3"], shared["bd3"] = _f32(W(pd[2][0])), bias1(pd[2][1])
    shared["wg1"], shared["bg1"] = _f32(W(pg[0][0])), bias1(pg[0][1])
    shared["wg2"], shared["bg2"] = _f32(W(pg[1][0])), bias1(pg[1][1])
    shared["wg3"], shared["bg3"] = _f32(W(pg[2][0])), bias1(pg[2][1])

    wo1 = W(po[0][0])
    shared["wox"] = qrep(kron4(wo1[0:8]))
    shared["woy"] = tile4(wo1[8:16])
    shared["woz"] = tile4(wo1[16:24])
    shared["bo1"] = bias4(po[0][1])
    shared["wo2"], shared["bo2"] = kron4(po[1][0]), bias4(po[1][1])
    shared["wo3"], shared["bo3"] = kron4(po[2][0]), bias4(po[2][1])

    wp1 = W(pp[0][0])
    shared["wpl"] = kron4(wp1[0:1])
    shared["wpy"] = tile4(wp1[1:9])
    shared["wpz"] = tile4(wp1[9:17])
    shared["bp1"] = bias4(pp[0][1])
    shared["wp2"], shared["bp2"] = kron4(pp[1][0]), bias4(pp[1][1])
    shared["wp3"], shared["bp3"] = kron4(pp[2][0]), bias4(pp[2][1])

    r = np.arange(cfg.JPC * cfg.WPAD)
    shared["limits"] = _f32(
        (r % cfg.WPAD).astype(np.float32).reshape(cfg.MPRL, 4).T)

    # selx: [128, 20] — px rows live at 32q + i*5 + ch (padded quarters)
    selx = np.zeros((128, 20), np.float32)
    for q in range(4):
        for i in range(4):
            for ch in range(5):
                selx[32 * q + i * 5 + ch, 5 * q + ch] = 1.0
    shared["selx"] = _f32(selx)
    shared["selxn"] = _f32(np.kron(np.eye(4), np.kron(np.ones((4, 1)), np.eye(8))))
    shared["triu"] = _f32(np.triu(np.ones((128, 128)), k=1))
    shared["id128"] = _f32(np.eye(128))
    shared["id4"] = _f32(np.tile(np.eye(32, dtype=np.float32), (4, 1)))
    shared["ones1"] = _f32(np.ones((1, 128)))
    shared["onesc"] = _f32(np.ones((128, 1)))
    shared["one11"] = _f32(np.ones((1, 1)))

    return shared, per_core


# ---------------------------------------------------------------------------
# Bass builder
# ---------------------------------------------------------------------------

QNAMES = ["qPoolDynamic", "qPoolDynamic1", "qPoolDynamic2",
          "qPoolDynamic3"]


def build(cfg, shared_arrs, dbg=False):
    nc = bacc.Bacc("TRN2", target_bir_lowering=False, debug=False,
                   num_devices=cfg.NC)
    FQ, KCH, NCH = cfg.FQ, cfg.KCH, cfg.NCHUNK
    NF = min(512, FQ)
    NFC = FQ // NF
    RJ = cfg.OPJ // 4          # free elems per job per row
    CCQ = cfg.CCQ

    io = {}

    def param(name, shape, dtype=FP):
        io[name] = nc.declare_dram_parameter(name, list(shape), dtype,
                                             isOutput=False)
        return io[name]

    param("xp", (128, FQ))
    param("xe", (128, cfg.FQE))
    param("epa", (128, cfg.FQE), I32)
    param("epb", (128, cfg.FQE), I32)
    param("bstart", (128, cfg.NB * 4), I32)
    param("bend", (128, cfg.NB * 4), I32)
    param("ipa", (128, FQ), I32)
    param("ipb", (128, FQ), I32)
    for name, arr in shared_arrs.items():
        param(name, arr.shape)

    ops_out = nc.declare_dram_parameter("ops_out", [16, FQ], FP, isOutput=True)
    prl_out = nc.declare_dram_parameter("prl_out", [4, cfg.MPRL], FP,
                                        isOutput=True)
    dbg_t = {}
    if dbg:
        for nm in ["dbg_dinv", "dbg_g", "dbg_aggr"]:
            dbg_t[nm] = nc.declare_dram_parameter(nm, [128, FQ], FP,
                                                  isOutput=True)
        dbg_t["dbg_y"] = nc.declare_dram_parameter("dbg_y", [8, cfg.JPC], FP,
                                                   isOutput=True)
        dbg_t["dbg_msg"] = nc.declare_dram_parameter(
            "dbg_msg", [128, KCH, 8], FP, isOutput=True)
        dbg_t["dbg_raw"] = nc.declare_dram_parameter(
            "dbg_raw", [128, KCH, 8], FP, isOutput=True)
        dbg_t["dbg_raw2"] = nc.declare_dram_parameter(
            "dbg_raw2", [128, KCH, 8], FP, isOutput=True)

    replica = [list(range(cfg.NC))]

    with tile.TileContext(nc) as tc:
        with (
            tc.tile_pool(name="dram", bufs=1, space="DRAM") as dpool,
            tc.tile_pool(name="const", bufs=1) as cpool,
            tc.tile_pool(name="big", bufs=1) as bigpool,
            tc.tile_pool(name="work", bufs=2) as wpool,
            tc.tile_pool(name="small", bufs=1) as spool,
            tc.tile_pool(name="msg", bufs=2) as mpool,
            tc.tile_pool(name="psA", bufs=2, space="PSUM") as psA,
            tc.tile_pool(name="psB", bufs=2, space="PSUM") as psB,
            tc.tile_pool(name="psC", bufs=2, space="PSUM") as psC,
            tc.tile_pool(name="psS", bufs=2, space="PSUM") as psS,
        ):
            ge = dpool.tile([cfg.EPC, 8], FP, name="ge")
            cum = dpool.tile([cfg.EPC + 1, 8], FP, name="cum")

            sb = {}
            for name, arr in shared_arrs.items():
                if name == "limits":
                    continue
                t = cpool.tile(list(arr.shape), FP, name=f"c_{name}")
                nc.sync.dma_start(out=t[:], in_=io[name][:])
                sb[name] = t

            def mlp3(rhs0, wa_ap, ba, wb, bb, wc, bc, m3, nf, tp=(0, 0)):
                ps1 = psA.tile([128, nf], FP, name="ps1", tag="A")
                nc.tensor.matmul(out=ps1[:], lhsT=wa_ap, rhs=rhs0,
                                 start=True, stop=True, tile_position=tp)
                a1 = wpool.tile([128, nf], FP, name="a1", tag="a1")
                nc.scalar.activation(out=a1[:], in_=ps1[:], func=AF.Relu,
                                     bias=sb[ba][:, 0:1])
                ps2 = psB.tile([64, nf], FP, name="ps2", tag="B")
                nc.tensor.matmul(out=ps2[:], lhsT=sb[wb][:], rhs=a1[:],
                                 start=True, stop=True)
                a2 = wpool.tile([64, nf], FP, name="a2", tag="a2")
                nc.scalar.activation(out=a2[:], in_=ps2[:], func=AF.Relu,
                                     bias=sb[bb][:, 0:1])
                ps3 = psC.tile([m3, nf], FP, name="ps3", tag="C")
                nc.tensor.matmul(out=ps3[:], lhsT=sb[wc][:], rhs=a2[:],
                                 start=True, stop=True)
                outc = wpool.tile([m3, nf], FP, name="outc", tag="t2k")
                nc.scalar.activation(out=outc[:], in_=ps3[:], func=AF.Identity,
                                     bias=sb[bc][:, 0:1])
                return outc

            # ---- Phase A: per-edge g_e = dinv[col]*mlp1(x[col]) -> ge ----
            FSE = cfg.FSE
            assert cfg.FQE % FSE == 0 and FSE % NF == 0
            TBE = next(t for t in (4, 2, 1) if FSE % (128 * t) == 0)
            for sec in range(cfg.FQE // FSE):
                s0 = sec * FSE
                ss = slice(s0, s0 + FSE)
                xe_s = bigpool.tile([128, FSE], FP, name="xe_s", tag="T4")
                nc.sync.dma_start(out=xe_s[:], in_=io["xe"][:, ss])
                ea_s = bigpool.tile([128, FSE], I32, name="ea_s", tag="T1")
                eb_s = bigpool.tile([128, FSE], I32, name="eb_s", tag="T2")
                nc.sync.dma_start(out=ea_s[:], in_=io["epa"][:, ss])
                nc.sync.dma_start(out=eb_s[:], in_=io["epb"][:, ss])
                w_s = bigpool.tile([128, FSE], FP, name="w_s", tag="T3")
                nc.vector.tensor_tensor(out=eb_s[:], in0=eb_s[:], in1=ea_s[:],
                                        op=OP.subtract)
                nc.vector.tensor_copy(out=w_s[:], in_=eb_s[:])
                nc.vector.tensor_scalar(out=w_s[:], in0=w_s[:], scalar1=1.0,
                                        scalar2=None, op0=OP.add)
                nc.scalar.sqrt(out=w_s[:], in_=w_s[:])
                nc.vector.reciprocal(out=w_s[:], in_=w_s[:])
                he_s = bigpool.tile([128, FSE], FP, name="he_s", tag="enq")
                for q in range(4):
                    for f in range(FSE // NF):
                        fs2 = slice(f * NF, (f + 1) * NF)
                        gfs = slice(s0 + f * NF, s0 + (f + 1) * NF)
                        hc = mlp3(xe_s[32 * q:32 * q + 20, fs2],
                                  sb["w1a"][32 * q:32 * q + 20, :],
                                  "b1a", "w1b", "b1b", "w1c", "b1c", 32, NF,
                                  tp=(32 * q, 0))
                        nc.sync.dma_start(out=he_s[32 * q:32 * q + 32, fs2],
                                          in_=hc[:])
                nc.vector.tensor_tensor(out=he_s[:], in0=he_s[:], in1=w_s[:],
                                        op=OP.mult)
                for q in range(4):
                    for cb in range(FSE // (128 * TBE)):
                        pst = psC.tile([128, 32 * TBE], FP, name="pse",
                                       tag="C")
                        for t in range(TBE):
                            f0 = cb * 128 * TBE + t * 128
                            nc.tensor.transpose(
                                out=pst[:, 32 * t:32 * t + 32],
                                in_=he_s[32 * q:32 * q + 32, f0:f0 + 128],
                                identity=sb["id4"][32 * q:32 * q + 32, :],
                                tile_position=(32 * q, 0))
                        gt = wpool.tile([128, 32 * TBE], FP, name="gte",
                                        tag="t2k")
                        nc.vector.tensor_copy(out=gt[:], in_=pst[:])
                        node0 = 4 * (cfg.FQE * q + s0 + cb * 128 * TBE)
                        nc.sync.dma_start(
                            out=ge[node0:node0 + 512 * TBE, :].rearrange(
                                "(t j i) d -> j t i d", t=TBE, j=128, i=4),
                            in_=gt[:])

            # ---- Phase 1: dinv [128, FQ] (tag T3) ----
            ipa_s = bigpool.tile([128, FQ], I32, name="ipa_s", tag="T1")
            ipb_s = bigpool.tile([128, FQ], I32, name="ipb_s", tag="T2")
            nc.sync.dma_start(out=ipa_s[:], in_=io["ipa"][:])
            nc.sync.dma_start(out=ipb_s[:], in_=io["ipb"][:])
            nc.vector.tensor_tensor(out=ipb_s[:], in0=ipb_s[:], in1=ipa_s[:],
                                    op=OP.subtract)
            dinv = bigpool.tile([128, FQ], FP, name="dinv", tag="T3")
            nc.vector.tensor_copy(out=dinv[:], in_=ipb_s[:])
            nc.vector.tensor_scalar(out=dinv[:], in0=dinv[:], scalar1=1.0,
                                    scalar2=None, op0=OP.add)
            nc.scalar.sqrt(out=dinv[:], in_=dinv[:])
            nc.vector.reciprocal(out=dinv[:], in_=dinv[:])
            if dbg:
                nc.sync.dma_start(out=dbg_t["dbg_dinv"][:], in_=dinv[:])

            # ---- Phase 2: mlp1 -> h (tag T1); g = dinv*h in place ----
            xp_s = bigpool.tile([128, FQ], FP, name="xp_s", tag="T4")
            nc.sync.dma_start(out=xp_s[:], in_=io["xp"][:])
            h = bigpool.tile([128, FQ], FP, name="h", tag="T1")


            for q in range(4):
                for f in range(NFC):
                    fs = slice(f * NF, (f + 1) * NF)
                    hc = mlp3(xp_s[32 * q:32 * q + 20, fs],
                              sb["w1a"][32 * q:32 * q + 20, :],
                              "b1a", "w1b", "b1b", "w1c", "b1c", 32, NF,
                              tp=(32 * q, 0))
                    nc.sync.dma_start(out=h[32 * q:32 * q + 32, fs], in_=hc[:])
            # px (x pooling) before xp slot is reused in phase 5
            px = spool.tile([128, cfg.JQ], FP, name="px")
            nc.vector.tensor_reduce(
                out=px[:], in_=xp_s[:].rearrange("p (j r) -> p j r", r=RJ),
                axis=AX.X, op=OP.add)
            # g = h * dinv (in place; tile "h" now holds g)
            nc.vector.tensor_tensor(out=h[:], in0=h[:], in1=dinv[:], op=OP.mult)
            g = h
            if dbg:
                nc.sync.dma_start(out=dbg_t["dbg_g"][:], in_=g[:])

            # ---- Phase 4: gather + scan + cum ----
            zrow = spool.tile([1, 8], FP, name="zrow")
            nc.vector.memset(zrow[:], 0.0)
            nc.sync.dma_start(out=cum[0:1, :], in_=zrow[:])
            gc0 = spool.tile([1, 8], FP, name="gc0")
            nc.vector.memset(gc0[:], 0.0)
            for c in range(NCH):
                msg = mpool.tile([128, KCH, 8], FP, name="msg", tag="msg")
                nc.sync.dma_start(
                    out=msg[:].rearrange("p k d -> p (k d)"),
                    in_=ge[c * cfg.CH:(c + 1) * cfg.CH, :].rearrange(
                        "(p k) d -> p (k d)", p=128))
                if dbg and c == NCH - 1:
                    nc.sync.dma_start(out=dbg_t["dbg_raw"][:], in_=msg[:])
                    last_msg = msg
                sc = mpool.tile([128, KCH, 8], FP, name="sc", tag="scan",
                                bufs=1)
                for chn in range(8):
                    nc.vector.tensor_tensor_scan(
                        out=sc[:, :, chn], data0=msg[:, :, chn],
                        data1=msg[:, :, chn], initial=0.0,
                        op0=OP.add, op1=OP.bypass)
                tots = wpool.tile([128, 8], FP, name="tots", tag="tots")
                nc.vector.tensor_copy(out=tots[:], in_=sc[:, KCH - 1, :])
                cps = psS.tile([128, 8], FP, name="cps", tag="S")
                nc.tensor.matmul(out=cps[:], lhsT=sb["triu"][:], rhs=tots[:],
                                 start=True, stop=False)
                nc.tensor.matmul(out=cps[:], lhsT=sb["ones1"][:],
                                 rhs=gc0[:], start=False, stop=True)
                ca = wpool.tile([128, 8], FP, name="ca", tag="ca")
                nc.vector.tensor_copy(out=ca[:], in_=cps[:])
                gps = psS.tile([1, 8], FP, name="gps", tag="S")
                nc.tensor.matmul(out=gps[:], lhsT=sb["onesc"][:], rhs=tots[:],
                                 start=True, stop=False)
                nc.tensor.matmul(out=gps[:], lhsT=sb["one11"][:], rhs=gc0[:],
                                 start=False, stop=True)
                nc.vector.tensor_copy(out=gc0[:], in_=gps[:])
                nc.vector.tensor_tensor(
                    out=sc[:], in0=sc[:],
                    in1=ca[:].unsqueeze(1).broadcast_to([128, KCH, 8]),
                    op=OP.add)
                nc.sync.dma_start(
                    out=cum[1 + c * cfg.CH: 1 + (c + 1) * cfg.CH, :].rearrange(
                        "(p k) d -> p (k d)", p=128),
                    in_=sc[:].rearrange("p k d -> p (k d)"))
                if dbg and c == NCH - 1:
                    nc.sync.dma_start(out=dbg_t["dbg_msg"][:], in_=sc[:])

            # ---- Phase 5: boundary gathers + diff -> aggr (pack4) ----
            aggr = bigpool.tile([128, FQ], FP, name="aggr", tag="T2")
            TB2 = min(4, CCQ)
            for q in range(4):
                qs = slice(q * CCQ * 4, (q + 1) * CCQ * 4)
                bs_q = mpool.tile([128, CCQ * 4], I32, name="bs_q", tag="colt")
                be_q = mpool.tile([128, CCQ * 4], I32, name="be_q", tag="colt")
                nc.sync.dma_start(out=bs_q[:], in_=io["bstart"][:, qs])
                nc.sync.dma_start(out=be_q[:], in_=io["bend"][:, qs])
                stq = bigpool.tile([128, CCQ * 4, 8], FP, name="stq", tag="T4")
                enq = bigpool.tile([128, CCQ * 4, 8], FP, name="enq", tag="enq")
                for jj in range(CCQ * 4):
                    nc.gpsimd.indirect_dma_start(
                        out=stq[:, jj, :], out_offset=None, in_=cum[:],
                        in_offset=bass.IndirectOffsetOnAxis(
                            ap=bs_q[:, jj:jj + 1], axis=0))
                    nc.gpsimd.indirect_dma_start(
                        out=enq[:, jj, :], out_offset=None, in_=cum[:],
                        in_offset=bass.IndirectOffsetOnAxis(
                            ap=be_q[:, jj:jj + 1], axis=0))
                nc.vector.tensor_tensor(
                    out=enq[:].rearrange("p b d -> p (b d)"),
                    in0=enq[:].rearrange("p b d -> p (b d)"),
                    in1=stq[:].rearrange("p b d -> p (b d)"), op=OP.subtract)
                for cb in range(CCQ // TB2):
                    pst = psC.tile([32, 128 * TB2], FP, name="psa", tag="C")
                    for t in range(TB2):
                        lc = cb * TB2 + t
                        nc.tensor.transpose(
                            out=pst[:, 128 * t:128 * t + 128],
                            in_=enq[:, 4 * lc:4 * lc + 4, :].rearrange(
                                "p i d -> p (i d)"),
                            identity=sb["id128"][:])
                    at = wpool.tile([32, 128 * TB2], FP, name="at",
                                    tag="t2k")
                    nc.vector.tensor_copy(out=at[:], in_=pst[:])
                    f0 = cb * TB2 * 128
                    nc.sync.dma_start(
                        out=aggr[32 * q:32 * q + 32, f0:f0 + 128 * TB2],
                        in_=at[:])

            nc.vector.tensor_tensor(out=aggr[:], in0=aggr[:], in1=g[:],
                                    op=OP.add)
            nc.vector.tensor_tensor(out=aggr[:], in0=aggr[:], in1=dinv[:],
                                    op=OP.mult)
            if dbg:
                nc.sync.dma_start(out=dbg_t["dbg_aggr"][:], in_=aggr[:])

            # ---- Phase 6: mlp2 -> x_node (tag T1, reuses g/h slot) ----
            xn = bigpool.tile([128, FQ], FP, name="xn", tag="T1")
            for q in range(4):
                for f in range(NFC):
                    fs = slice(f * NF, (f + 1) * NF)
                    xc = mlp3(aggr[32 * q:32 * q + 32, fs],
                              sb["w2a"][32 * q:32 * q + 32, :],
                              "b2a", "w2b", "b2b", "w2c", "b2c", 32, NF,
                              tp=(32 * q, 0))
                    nc.sync.dma_start(out=xn[32 * q:32 * q + 32, fs], in_=xc[:])

            # ---- Phase 7: pooling + mlp_dag + mlp_global ----
            pxn = spool.tile([128, cfg.JQ], FP, name="pxn")
            nc.vector.tensor_reduce(
                out=pxn[:], in_=xn[:].rearrange("p (j r) -> p j r", r=RJ),
                axis=AX.X, op=OP.add)
            sxp = psS.tile([5, cfg.JPC], FP, name="sxp", tag="S")
            sxnp = psS.tile([8, cfg.JPC], FP, name="sxnp", tag="S")
            for q in range(4):
                js = slice(q * cfg.JQ, (q + 1) * cfg.JQ)
                nc.tensor.matmul(out=sxp[:, js],
                                 lhsT=sb["selx"][:, 5 * q:5 * q + 5],
                                 rhs=px[:], start=True, stop=True)
                nc.tensor.matmul(out=sxnp[:, js],
                                 lhsT=sb["selxn"][:, 8 * q:8 * q + 8],
                                 rhs=pxn[:], start=True, stop=True)
            sx = spool.tile([5, cfg.JPC], FP, name="sx_s")
            sxn = spool.tile([8, cfg.JPC], FP, name="sxn_s")
            nc.vector.tensor_copy(out=sx[:], in_=sxp[:])
            nc.vector.tensor_copy(out=sxn[:], in_=sxnp[:])

            def mlp3s(rhs_list, ws, bs_, ms, name):
                cols_n = rhs_list[0][1].shape[-1]
                ps = psS.tile([ms[0], cols_n], FP, name=f"{name}p1", tag="S")
                for j, (wn, rap) in enumerate(rhs_list):
                    nc.tensor.matmul(out=ps[:], lhsT=sb[wn][:], rhs=rap,
                                     start=(j == 0),
                                     stop=(j == len(rhs_list) - 1))
                o1 = spool.tile([ms[0], cols_n], FP, name=f"{name}o1")
                nc.scalar.activation(out=o1[:], in_=ps[:], func=AF.Relu,
                                     bias=sb[bs_[0]][:, 0:1])
                ps2 = psS.tile([ms[1], cols_n], FP, name=f"{name}p2", tag="S")
                nc.tensor.matmul(out=ps2[:], lhsT=sb[ws[1]][:], rhs=o1[:],
                                 start=True, stop=True)
                o2 = spool.tile([ms[1], cols_n], FP, name=f"{name}o2")
                nc.scalar.activation(out=o2[:], in_=ps2[:], func=AF.Relu,
                                     bias=sb[bs_[1]][:, 0:1])
                ps3 = psS.tile([ms[2], cols_n], FP, name=f"{name}p3", tag="S")
                nc.tensor.matmul(out=ps3[:], lhsT=sb[ws[2]][:], rhs=o2[:],
                                 start=True, stop=True)
                o3 = spool.tile([ms[2], cols_n], FP, name=f"{name}o3")
                nc.scalar.activation(out=o3[:], in_=ps3[:], func=AF.Identity,
                                     bias=sb[bs_[2]][:, 0:1])
                return o3

            y = mlp3s([("wdx", sx[:]), ("wdxn", sxn[:])],
                      [None, "wd2", "wd3"], ["bd1", "bd2", "bd3"],
                      [32, 16, 8], "dag")
            if dbg:
                nc.sync.dma_start(out=dbg_t["dbg_y"][:], in_=y[:])
                nc.sync.dma_start(out=dbg_t["dbg_raw2"][:], in_=last_msg[:])
            yz = spool.tile([8, cfg.EVC], FP, name="yz")
            nc.vector.tensor_reduce(
                out=yz[:], in_=y[:].rearrange("p (e j) -> p e j", j=cfg.JPE),
                axis=AX.X, op=OP.add)
            z = mlp3s([("wg1", yz[:])],
                      [None, "wg2", "wg3"], ["bg1", "bg2", "bg3"],
                      [32, 16, 8], "glb")

            # ---- Phase 8: mlp_op -> ops_out ----
            JC = NF // RJ               # jobs per free-chunk
            EPN = cfg.OPJ * cfg.JPE // 4  # free elems per env per row
            for q in range(4):
                for f in range(NFC):
                    fs = slice(f * NF, (f + 1) * NF)
                    j0 = q * cfg.JQ + f * JC
                    e0 = (q * FQ + f * NF) // EPN
                    yr = wpool.tile([8, NF], FP, name="yr", tag="yr")
                    nc.vector.tensor_copy(
                        out=yr[:].rearrange("p (j r) -> p j r", r=RJ),
                        in_=y[:, j0:j0 + JC].unsqueeze(2).broadcast_to(
                            [8, JC, RJ]))
                    zr = wpool.tile([8, NF], FP, name="zr", tag="zr")
                    if EPN >= NF:
                        nc.vector.tensor_copy(
                            out=zr[:],
                            in_=z[:, e0:e0 + 1].broadcast_to([8, NF]))
                    else:
                        ec_n = NF // EPN
                        nc.vector.tensor_copy(
                            out=zr[:].rearrange("p (e r) -> p e r", r=EPN),
                            in_=z[:, e0:e0 + ec_n].unsqueeze(2).broadcast_to(
                                [8, ec_n, EPN]))
                    ps1 = psA.tile([128, NF], FP, name="po1", tag="A")
                    nc.tensor.matmul(out=ps1[:],
                                     lhsT=sb["wox"][32 * q:32 * q + 32, :],
                                     rhs=xn[32 * q:32 * q + 32, fs],
                                     start=True, stop=False,
                                     tile_position=(32 * q, 0))
                    nc.tensor.matmul(out=ps1[:], lhsT=sb["woy"][:], rhs=yr[:],
                                     start=False, stop=False)
                    nc.tensor.matmul(out=ps1[:], lhsT=sb["woz"][:], rhs=zr[:],
                                     start=False, stop=True)
                    a1 = wpool.tile([128, NF], FP, name="oa1", tag="a1")
                    nc.scalar.activation(out=a1[:], in_=ps1[:], func=AF.Relu,
                                         bias=sb["bo1"][:, 0:1])
                    ps2 = psB.tile([64, NF], FP, name="po2", tag="B")
                    nc.tensor.matmul(out=ps2[:], lhsT=sb["wo2"][:], rhs=a1[:],
                                     start=True, stop=True)
                    a2 = wpool.tile([64, NF], FP, name="oa2", tag="a2")
                    nc.scalar.activation(out=a2[:], in_=ps2[:], func=AF.Relu,
                                         bias=sb["bo2"][:, 0:1])
                    ps3 = psC.tile([4, NF], FP, name="po3", tag="C")
                    nc.tensor.matmul(out=ps3[:], lhsT=sb["wo3"][:], rhs=a2[:],
                                     start=True, stop=True)
                    oc = wpool.tile([4, NF], FP, name="oc", tag="t2k")
                    nc.scalar.activation(out=oc[:], in_=ps3[:],
                                         func=AF.Identity,
                                         bias=sb["bo3"][:, 0:1])
                    nc.sync.dma_start(out=ops_out[4 * q:4 * q + 4, fs],
                                      in_=oc[:])

            # ---- Phase 9: prlvl -> prl_out ----
            RW = cfg.WPAD // 4
            REW = cfg.WPAD * cfg.JPE // 4
            lim_s = bigpool.tile([4, cfg.MPRL], FP, name="lim_s", tag="T4")
            nc.sync.dma_start(out=lim_s[:], in_=io["limits"][:])
            yrp = bigpool.tile([8, cfg.MPRL], FP, name="yrp", tag="T3")
            nc.vector.tensor_copy(
                out=yrp[:].rearrange("p (j r) -> p j r", r=RW),
                in_=y[:].unsqueeze(2).broadcast_to([8, cfg.JPC, RW]))
            zrp = bigpool.tile([8, cfg.MPRL], FP, name="zrp", tag="enq")
            nc.vector.tensor_copy(
                out=zrp[:].rearrange("p (e r) -> p e r", r=REW),
                in_=z[:].unsqueeze(2).broadcast_to([8, cfg.EVC, REW]))
            f0 = 0
            while f0 < cfg.MPRL:
                nf = min(512, cfg.MPRL - f0)
                fs = slice(f0, f0 + nf)
                ps1 = psA.tile([128, nf], FP, name="pp1", tag="A")
                nc.tensor.matmul(out=ps1[:], lhsT=sb["wpl"][:],
                                 rhs=lim_s[:, fs], start=True, stop=False)
                nc.tensor.matmul(out=ps1[:], lhsT=sb["wpy"][:], rhs=yrp[:, fs],
                                 start=False, stop=False)
                nc.tensor.matmul(out=ps1[:], lhsT=sb["wpz"][:], rhs=zrp[:, fs],
                                 start=False, stop=True)
                a1 = wpool.tile([128, nf], FP, name="pa1", tag="a1")
                nc.scalar.activation(out=a1[:], in_=ps1[:], func=AF.Relu,
                                     bias=sb["bp1"][:, 0:1])
                ps2 = psB.tile([64, nf], FP, name="pp2", tag="B")
                nc.tensor.matmul(out=ps2[:], lhsT=sb["wp2"][:], rhs=a1[:],
                                 start=True, stop=True)
                a2 = wpool.tile([64, nf], FP, name="pa2", tag="a2")
                nc.scalar.activation(out=a2[:], in_=ps2[:], func=AF.Relu,
                                     bias=sb["bp2"][:, 0:1])
                ps3 = psC.tile([4, nf], FP, name="pp3", tag="C")
                nc.tensor.matmul(out=ps3[:], lhsT=sb["wp3"][:], rhs=a2[:],
                                 start=True, stop=True)
                oc = wpool.tile([4, nf], FP, name="poc", tag="t2k")
                nc.scalar.activation(out=oc[:], in_=ps3[:], func=AF.Identity,
                                     bias=sb["bp3"][:, 0:1])
                nc.sync.dma_start(out=prl_out[:, fs], in_=oc[:])
                f0 += nf

    nc.compile()
    return nc


# ---------------------------------------------------------------------------
# Host postprocessing
# ---------------------------------------------------------------------------

def host_post(cfg, results, batch, num_jobs_per_env):
    ops = []
    prl = []
    for c in range(cfg.NC):
        b = np.asarray(results[c]["ops_out"])          # [16, FQ]
        t = b.reshape(4, 4, cfg.FQ).transpose(0, 2, 1).reshape(cfg.NPC)
        ops.append(t)
        p = np.asarray(results[c]["prl_out"])          # [4, MPRL]
        rows = p.T.reshape(cfg.JPC * cfg.WPAD)
        prl.append(rows.reshape(cfg.JPC, cfg.WPAD)[:, :cfg.W1])
    op_scores = np.concatenate(ops).astype(np.float32)
    prlvl = np.concatenate(prl, axis=0).astype(np.float32)

    njpe = np.asarray(num_jobs_per_env)
    total_jobs = int(njpe.sum())
    batch = np.asarray(batch)
    nopj = np.bincount(batch, minlength=total_jobs).astype(np.int32)
    env_ids = np.repeat(np.arange(njpe.shape[0]), njpe)
    nope = np.zeros(njpe.shape[0], dtype=np.int32)
    np.add.at(nope, env_ids, nopj)
    job_indptr = np.concatenate([np.zeros(1, np.int32),
                                 np.cumsum(njpe).astype(np.int32)])
    return op_scores, prlvl, nope, job_indptr


# ---------------------------------------------------------------------------
# Entry points
# ---------------------------------------------------------------------------

_CACHE = {}


def get_nc(cfg, shared):
    if cfg not in _CACHE:
        _CACHE[cfg] = build(cfg, shared)
    return _CACHE[cfg]


def make_in_maps(cfg, shared, per_core):
    in_maps = []
    for c in range(cfg.NC):
        m = dict(shared)
        m.update(per_core[c])
        in_maps.append(m)
    return in_maps


def run(cfg, x, edge_index, batch, num_jobs_per_env, n_workers, params,
        trace=False):
    shared, per_core = host_prep(cfg, x, edge_index, params)
    nc = get_nc(cfg, shared)
    in_maps = make_in_maps(cfg, shared, per_core)
    res = run_bass_kernel_spmd(nc, in_maps, core_ids=list(range(cfg.NC)),
                               trace=trace)
    out = host_post(cfg, res.results, batch, num_jobs_per_env)
    return out, res


def kernel(x, edge_index, batch, num_jobs_per_env, n_workers, params):
    cfg = FULL
    assert int(n_workers) == cfg.NW
    out, _ = run(cfg, x, edge_index, batch, num_jobs_per_env, n_workers,
                 params)
    return out


# revision 37
# speedup vs baseline: 2.9624x; 1.1812x over previous
"""Trainium2 (8 NeuronCore) Bass kernel for the ActorNetwork GNN.

Self-contained: hardcodes the reference's static structure
(N=1048576 nodes, 8.4M edges, 2048 jobs x 512 ops, 64 envs x 32 jobs, 50 workers).

Host side does only integer index preprocessing (sorts / searchsorted / layout
permutation of inputs) and final unpermute/concat of outputs; all float math of
the network runs on the 8 NeuronCores.

Sharding: nodes in 8 contiguous ranges of 131072; edges dest(row)-sorted and
split at node boundaries. GCN aggregation:
  deg[v] = colcount(v)+1,  dinv = deg^-1/2,  g = dinv * mlp1(x)
  aggr[r] = dinv[r] * (sum_{e:row=r} g[col_e] + g[r])
g is AllGathered (32MB) so each core serves its own edges' gathers locally via
SWDGE indirect DMA; segment sums via prefix-scan over dest-sorted messages +
boundary-difference gathers (host-precomputed indptr grids).

SBUF layout: "pack4 quarter" — node v_local = 4*(FQ*q + f) + i maps to
partition p = q*4C + i*C + ch, free f (C = channels of that table).
"""

import sys
import numpy as np
from dataclasses import dataclass

sys.path.insert(0, "/opt/trn_rl_repo")

import concourse.bass as bass
import concourse.mybir as mybir
import concourse.tile as tile
from concourse import bacc
from concourse.bass_utils import run_bass_kernel_spmd

FP = mybir.dt.float32
I32 = mybir.dt.int32
AX = mybir.AxisListType
OP = mybir.AluOpType
AF = mybir.ActivationFunctionType


@dataclass(frozen=True)
class Cfg:
    NC: int = 8          # cores
    NPC: int = 131072    # nodes per core (16 * FQ)
    OPJ: int = 512       # ops per job
    JPE: int = 32        # jobs per env
    KCH: int = 256       # free size of one edge-chunk row (chunk = 128*KCH)
    NCHUNK: int = 33     # edge chunks per core (padded)
    NW: int = 50         # n_workers

    @property
    def N(self):
        return self.NC * self.NPC

    @property
    def E(self):
        return 8 * self.N

    @property
    def FQ(self):
        return self.NPC // 16

    @property
    def CH(self):
        return 128 * self.KCH

    @property
    def EPC(self):
        return self.NCHUNK * self.CH

    @property
    def JPC(self):
        return self.NPC // self.OPJ

    @property
    def EVC(self):
        return self.JPC // self.JPE

    @property
    def JQ(self):
        return self.FQ // (self.OPJ // 4)   # jobs per quarter

    @property
    def W1(self):
        return self.NW + 1

    @property
    def WPAD(self):
        return ((self.W1 + 3) // 4) * 4

    @property
    def MPRL(self):
        return self.JPC * self.WPAD // 4

    @property
    def NB(self):
        return self.NPC // 512              # boundary-grid blocks

    @property
    def CCQ(self):
        return self.NB // 4                 # 128-node blocks per quarter

    @property
    def FQE(self):
        return self.EPC // 16               # pack4 width of the edge stream

    @property
    def FSE(self):
        return min(2048, self.FQE)          # per-section f width (edge mlp)


FULL = Cfg()

# HW consumes indirect-DMA offset grids column-major (descriptor d reads
# offsets[d % 128, d // 128]); CoreSim consumes them row-major. Host lays
# out offsets accordingly.
HW_OFFSET_ORDER = False


def _hw_off(grid):
    """[128, M] desired row-major offset grid -> layout the HW consumes."""
    if not HW_OFFSET_ORDER:
        return grid
    m = grid.shape[1]
    return np.ascontiguousarray(grid.reshape(128 * m)[
        np.arange(128 * m).reshape(128, m, order="F")])


def _f32(a):
    return np.ascontiguousarray(a, dtype=np.float32)


def _i32(a):
    return np.ascontiguousarray(a, dtype=np.int32)


# ---------------------------------------------------------------------------
# Host preprocessing
# ---------------------------------------------------------------------------

def host_prep(cfg, x, edge_index, params):
    N, NPC, NC, FQ = cfg.N, cfg.NPC, cfg.NC, cfg.FQ

    row = np.asarray(edge_index[0], dtype=np.int64)
    col = np.asarray(edge_index[1], dtype=np.int64)
    x = np.asarray(x, dtype=np.float32)

    order = np.argsort(row, kind="stable")
    rs = row[order]
    cs = col[order].astype(np.int32)
    bounds = np.searchsorted(rs, np.arange(0, N + 1, NPC))

    csort = np.sort(col)
    ipcol = np.searchsorted(csort, np.arange(N + 1)).astype(np.int64)

    shared = {}
    per_core = []

    def pack_rows(arr_v, C, W=FQ):
        # arr_v [16W, C] -> [16C, W], row p = q*4C + i*C + ch
        a = arr_v.reshape(4, W, 4, C)
        return a.transpose(0, 2, 3, 1).reshape(16 * C, W)

    def pack_rows_pad32(arr_v, C, W=FQ):
        # like pack_rows but each quarter block padded to 32 rows (PE base rule)
        a = arr_v.reshape(4, W, 4, C).transpose(0, 2, 3, 1)  # (q, i, ch, f)
        out = np.zeros((128, W), np.float32)
        for q in range(4):
            out[32 * q:32 * q + 4 * C] = a[q].reshape(4 * C, W)
        return out

    def expand8(arr16):
        # [16, FQ] -> [128, FQ] rows (q,i) replicated to (q,i,ch) for C=8
        return np.repeat(arr16, 8, axis=0)

    for c in range(NC):
        base = c * NPC
        e0, e1 = int(bounds[c]), int(bounds[c + 1])
        ec = e1 - e0
        assert ec <= cfg.EPC, f"core {c}: {ec} edges > EPC {cfg.EPC}"
        colpad = np.zeros(cfg.EPC, dtype=np.int64)
        colpad[:ec] = cs[e0:e1]
        # per-edge streams in pack4(FQE) layout: x[col_e], ipcol[col_e(+1)]
        xe = pack_rows_pad32(x[colpad], 5, cfg.FQE)
        epa = np.repeat(pack_rows(
            ipcol[colpad].reshape(cfg.EPC, 1), 1, cfg.FQE), 8, axis=0)
        epb = np.repeat(pack_rows(
            ipcol[colpad + 1].reshape(cfg.EPC, 1), 1, cfg.FQE), 8, axis=0)

        ip = np.searchsorted(rs[e0:e1], base + np.arange(NPC + 1)).astype(np.int32)

        def gridx(a):
            # per-quarter block-grid with ONE overlap cc-block appended:
            # grid[p, cc*4+i] = a[4*(cc*128+p)+i], cc in [0, CCQ]
            CQ4 = cfg.CCQ * 4
            parts = []
            av = a  # [NPC+1]
            for q in range(4):
                vbase = q * cfg.CCQ * 512
                g2 = np.zeros((128, CQ4 + 4), np.int32)
                for blk in range(cfg.CCQ + 1):
                    v = vbase + 4 * (blk * 128 +
                                     np.arange(128)[:, None]) + np.arange(4)
                    g2[:, blk * 4:blk * 4 + 4] = av[np.minimum(v, NPC)]
                parts.append(g2)
            return np.concatenate(parts, axis=1)

        vids = base + np.arange(NPC, dtype=np.int64)
        ipa = expand8(pack_rows(ipcol[vids].reshape(NPC, 1), 1)).astype(np.int32)
        ipb = expand8(pack_rows(ipcol[vids + 1].reshape(NPC, 1), 1)).astype(np.int32)

        per_core.append({
            "xp": _f32(pack_rows_pad32(x[base:base + NPC], 5)),
            "xe": _f32(xe),
            "epa": _i32(epa),
            "epb": _i32(epb),
            "bstart": _i32(gridx(ip)),
            "ipa": _i32(ipa),
            "ipb": _i32(ipb),
        })

    def W(p):
        return np.asarray(p, dtype=np.float32)

    def kron4(w):
        return _f32(np.kron(np.eye(4, dtype=np.float32), W(w)))

    def qrep(w):
        # [K, M] block-diag lhsT replicated into 4 base-32 quarter slots
        w = np.asarray(w, np.float32)
        out = np.zeros((128, w.shape[1]), np.float32)
        for q in range(4):
            out[32 * q:32 * q + w.shape[0]] = w
        return _f32(out)

    def tile4(w):
        return _f32(np.tile(W(w), (1, 4)))

    def bias4(b):
        return _f32(np.tile(W(b), 4).reshape(-1, 1))

    def bias1(b):
        return _f32(W(b).reshape(-1, 1))

    p1, p2 = params["mlp1"], params["mlp2"]
    pd, pg = params["mlp_dag"], params["mlp_global"]
    po, pp = params["mlp_op"], params["mlp_prlvl"]

    shared["w1a"], shared["w1b"], shared["w1c"] = (
        qrep(kron4(p1[0][0])), kron4(p1[1][0]), kron4(p1[2][0]))
    shared["b1a"], shared["b1b"], shared["b1c"] = (
        bias4(p1[0][1]), bias4(p1[1][1]), bias4(p1[2][1]))
    shared["w2a"], shared["w2b"], shared["w2c"] = (
        qrep(kron4(p2[0][0])), kron4(p2[1][0]), kron4(p2[2][0]))
    shared["b2a"], shared["b2b"], shared["b2c"] = (
        bias4(p2[0][1]), bias4(p2[1][1]), bias4(p2[2][1]))

    wd1 = W(pd[0][0])
    shared["wdx"], shared["wdxn"] = _f32(wd1[:5]), _f32(wd1[5:])
    shared["bd1"] = bias1(pd[0][1])
    shared["wd2"], shared["bd2"] = _f32(W(pd[1][0])), bias1(pd[1][1])
    shared["wd3"], shared["bd3"] = _f32(W(pd[2][0])), bias1(pd[2][1])
    shared["wg1"], shared["bg1"] = _f32(W(pg[0][0])), bias1(pg[0][1])
    shared["wg2"], shared["bg2"] = _f32(W(pg[1][0])), bias1(pg[1][1])
    shared["wg3"], shared["bg3"] = _f32(W(pg[2][0])), bias1(pg[2][1])

    wo1 = W(po[0][0])
    shared["wox"] = qrep(kron4(wo1[0:8]))
    shared["woy"] = tile4(wo1[8:16])
    shared["woz"] = tile4(wo1[16:24])
    shared["bo1"] = bias4(po[0][1])
    shared["wo2"], shared["bo2"] = kron4(po[1][0]), bias4(po[1][1])
    shared["wo3"], shared["bo3"] = kron4(po[2][0]), bias4(po[2][1])

    wp1 = W(pp[0][0])
    shared["wpl"] = kron4(wp1[0:1])
    shared["wpy"] = tile4(wp1[1:9])
    shared["wpz"] = tile4(wp1[9:17])
    shared["bp1"] = bias4(pp[0][1])
    shared["wp2"], shared["bp2"] = kron4(pp[1][0]), bias4(pp[1][1])
    shared["wp3"], shared["bp3"] = kron4(pp[2][0]), bias4(pp[2][1])

    r = np.arange(cfg.JPC * cfg.WPAD)
    shared["limits"] = _f32(
        (r % cfg.WPAD).astype(np.float32).reshape(cfg.MPRL, 4).T)

    # selx: [128, 20] — px rows live at 32q + i*5 + ch (padded quarters)
    selx = np.zeros((128, 20), np.float32)
    for q in range(4):
        for i in range(4):
            for ch in range(5):
                selx[32 * q + i * 5 + ch, 5 * q + ch] = 1.0
    shared["selx"] = _f32(selx)
    shared["selxn"] = _f32(np.kron(np.eye(4), np.kron(np.ones((4, 1)), np.eye(8))))
    shared["triu"] = _f32(np.triu(np.ones((128, 128)), k=1))
    shared["id128"] = _f32(np.eye(128))
    shared["id4"] = _f32(np.tile(np.eye(32, dtype=np.float32), (4, 1)))
    shared["ones1"] = _f32(np.ones((1, 128)))
    shared["onesc"] = _f32(np.ones((128, 1)))
    shared["one11"] = _f32(np.ones((1, 1)))

    return shared, per_core


# ---------------------------------------------------------------------------
# Bass builder
# ---------------------------------------------------------------------------

QNAMES = ["qPoolDynamic", "qPoolDynamic1", "qPoolDynamic2",
          "qPoolDynamic3"]


def build(cfg, shared_arrs, dbg=False):
    nc = bacc.Bacc("TRN2", target_bir_lowering=False, debug=False,
                   num_devices=cfg.NC)
    FQ, KCH, NCH = cfg.FQ, cfg.KCH, cfg.NCHUNK
    NF = min(512, FQ)
    NFC = FQ // NF
    RJ = cfg.OPJ // 4          # free elems per job per row
    CCQ = cfg.CCQ

    io = {}

    def param(name, shape, dtype=FP):
        io[name] = nc.declare_dram_parameter(name, list(shape), dtype,
                                             isOutput=False)
        return io[name]

    param("xp", (128, FQ))
    param("xe", (128, cfg.FQE))
    param("epa", (128, cfg.FQE), I32)
    param("epb", (128, cfg.FQE), I32)
    param("bstart", (128, (cfg.CCQ * 4 + 4) * 4), I32)
    param("ipa", (128, FQ), I32)
    param("ipb", (128, FQ), I32)
    for name, arr in shared_arrs.items():
        param(name, arr.shape)

    ops_out = nc.declare_dram_parameter("ops_out", [16, FQ], FP, isOutput=True)
    prl_out = nc.declare_dram_parameter("prl_out", [4, cfg.MPRL], FP,
                                        isOutput=True)
    dbg_t = {}
    if dbg:
        for nm in ["dbg_dinv", "dbg_g", "dbg_aggr"]:
            dbg_t[nm] = nc.declare_dram_parameter(nm, [128, FQ], FP,
                                                  isOutput=True)
        dbg_t["dbg_y"] = nc.declare_dram_parameter("dbg_y", [8, cfg.JPC], FP,
                                                   isOutput=True)
        dbg_t["dbg_msg"] = nc.declare_dram_parameter(
            "dbg_msg", [128, KCH, 8], FP, isOutput=True)
        dbg_t["dbg_raw"] = nc.declare_dram_parameter(
            "dbg_raw", [128, KCH, 8], FP, isOutput=True)
        dbg_t["dbg_raw2"] = nc.declare_dram_parameter(
            "dbg_raw2", [128, KCH, 8], FP, isOutput=True)

    replica = [list(range(cfg.NC))]

    with tile.TileContext(nc) as tc:
        with (
            tc.tile_pool(name="dram", bufs=1, space="DRAM") as dpool,
            tc.tile_pool(name="const", bufs=1) as cpool,
            tc.tile_pool(name="big", bufs=1) as bigpool,
            tc.tile_pool(name="work", bufs=2) as wpool,
            tc.tile_pool(name="small", bufs=1) as spool,
            tc.tile_pool(name="msg", bufs=2) as mpool,
            tc.tile_pool(name="psA", bufs=2, space="PSUM") as psA,
            tc.tile_pool(name="psB", bufs=2, space="PSUM") as psB,
            tc.tile_pool(name="psC", bufs=2, space="PSUM") as psC,
            tc.tile_pool(name="psS", bufs=2, space="PSUM") as psS,
        ):
            ge = dpool.tile([cfg.EPC, 8], FP, name="ge")
            cum = dpool.tile([cfg.EPC + 1, 8], FP, name="cum")

            sb = {}
            for name, arr in shared_arrs.items():
                if name == "limits":
                    continue
                t = cpool.tile(list(arr.shape), FP, name=f"c_{name}")
                nc.sync.dma_start(out=t[:], in_=io[name][:])
                sb[name] = t

            def mlp3(rhs0, wa_ap, ba, wb, bb, wc, bc, m3, nf, tp=(0, 0)):
                ps1 = psA.tile([128, nf], FP, name="ps1", tag="A")
                nc.tensor.matmul(out=ps1[:], lhsT=wa_ap, rhs=rhs0,
                                 start=True, stop=True, tile_position=tp)
                a1 = wpool.tile([128, nf], FP, name="a1", tag="a1")
                nc.scalar.activation(out=a1[:], in_=ps1[:], func=AF.Relu,
                                     bias=sb[ba][:, 0:1])
                ps2 = psB.tile([64, nf], FP, name="ps2", tag="B")
                nc.tensor.matmul(out=ps2[:], lhsT=sb[wb][:], rhs=a1[:],
                                 start=True, stop=True)
                a2 = wpool.tile([64, nf], FP, name="a2", tag="a2")
                nc.scalar.activation(out=a2[:], in_=ps2[:], func=AF.Relu,
                                     bias=sb[bb][:, 0:1])
                ps3 = psC.tile([m3, nf], FP, name="ps3", tag="C")
                nc.tensor.matmul(out=ps3[:], lhsT=sb[wc][:], rhs=a2[:],
                                 start=True, stop=True)
                outc = wpool.tile([m3, nf], FP, name="outc", tag="t2k")
                nc.scalar.activation(out=outc[:], in_=ps3[:], func=AF.Identity,
                                     bias=sb[bc][:, 0:1])
                return outc

            # ---- Phase A: per-edge g_e = dinv[col]*mlp1(x[col]) -> ge ----
            FSE = cfg.FSE
            assert cfg.FQE % FSE == 0 and FSE % NF == 0
            TBE = next(t for t in (4, 2, 1) if FSE % (128 * t) == 0)
            for sec in range(cfg.FQE // FSE):
                s0 = sec * FSE
                ss = slice(s0, s0 + FSE)
                xe_s = bigpool.tile([128, FSE], FP, name="xe_s", tag="T4")
                nc.sync.dma_start(out=xe_s[:], in_=io["xe"][:, ss])
                ea_s = bigpool.tile([128, FSE], I32, name="ea_s", tag="T1")
                eb_s = bigpool.tile([128, FSE], I32, name="eb_s", tag="T2")
                nc.sync.dma_start(out=ea_s[:], in_=io["epa"][:, ss])
                nc.sync.dma_start(out=eb_s[:], in_=io["epb"][:, ss])
                w_s = bigpool.tile([128, FSE], FP, name="w_s", tag="T3")
                nc.vector.tensor_tensor(out=eb_s[:], in0=eb_s[:], in1=ea_s[:],
                                        op=OP.subtract)
                nc.vector.tensor_copy(out=w_s[:], in_=eb_s[:])
                nc.vector.tensor_scalar(out=w_s[:], in0=w_s[:], scalar1=1.0,
                                        scalar2=None, op0=OP.add)
                nc.scalar.sqrt(out=w_s[:], in_=w_s[:])
                nc.vector.reciprocal(out=w_s[:], in_=w_s[:])
                he_s = bigpool.tile([128, FSE], FP, name="he_s", tag="enq")
                for q in range(4):
                    for f in range(FSE // NF):
                        fs2 = slice(f * NF, (f + 1) * NF)
                        gfs = slice(s0 + f * NF, s0 + (f + 1) * NF)
                        hc = mlp3(xe_s[32 * q:32 * q + 20, fs2],
                                  sb["w1a"][32 * q:32 * q + 20, :],
                                  "b1a", "w1b", "b1b", "w1c", "b1c", 32, NF,
                                  tp=(32 * q, 0))
                        nc.sync.dma_start(out=he_s[32 * q:32 * q + 32, fs2],
                                          in_=hc[:])
                nc.vector.tensor_tensor(out=he_s[:], in0=he_s[:], in1=w_s[:],
                                        op=OP.mult)
                for q in range(4):
                    for cb in range(FSE // (128 * TBE)):
                        pst = psC.tile([128, 32 * TBE], FP, name="pse",
                                       tag="C")
                        for t in range(TBE):
                            f0 = cb * 128 * TBE + t * 128
                            nc.tensor.transpose(
                                out=pst[:, 32 * t:32 * t + 32],
                                in_=he_s[32 * q:32 * q + 32, f0:f0 + 128],
                                identity=sb["id4"][32 * q:32 * q + 32, :],
                                tile_position=(32 * q, 0))
                        gt = wpool.tile([128, 32 * TBE], FP, name="gte",
                                        tag="t2k")
                        nc.vector.tensor_copy(out=gt[:], in_=pst[:])
                        node0 = 4 * (cfg.FQE * q + s0 + cb * 128 * TBE)
                        nc.sync.dma_start(
                            out=ge[node0:node0 + 512 * TBE, :].rearrange(
                                "(t j i) d -> j t i d", t=TBE, j=128, i=4),
                            in_=gt[:])

            # ---- Phase 1: dinv [128, FQ] (tag T3) ----
            ipa_s = bigpool.tile([128, FQ], I32, name="ipa_s", tag="T1")
            ipb_s = bigpool.tile([128, FQ], I32, name="ipb_s", tag="T2")
            nc.sync.dma_start(out=ipa_s[:], in_=io["ipa"][:])
            nc.sync.dma_start(out=ipb_s[:], in_=io["ipb"][:])
            nc.vector.tensor_tensor(out=ipb_s[:], in0=ipb_s[:], in1=ipa_s[:],
                                    op=OP.subtract)
            dinv = bigpool.tile([128, FQ], FP, name="dinv", tag="T3")
            nc.vector.tensor_copy(out=dinv[:], in_=ipb_s[:])
            nc.vector.tensor_scalar(out=dinv[:], in0=dinv[:], scalar1=1.0,
                                    scalar2=None, op0=OP.add)
            nc.scalar.sqrt(out=dinv[:], in_=dinv[:])
            nc.vector.reciprocal(out=dinv[:], in_=dinv[:])
            if dbg:
                nc.sync.dma_start(out=dbg_t["dbg_dinv"][:], in_=dinv[:])

            # ---- Phase 2: mlp1 -> h (tag T1); g = dinv*h in place ----
            xp_s = bigpool.tile([128, FQ], FP, name="xp_s", tag="T4")
            nc.sync.dma_start(out=xp_s[:], in_=io["xp"][:])
            h = bigpool.tile([128, FQ], FP, name="h", tag="T1")


            for q in range(4):
                for f in range(NFC):
                    fs = slice(f * NF, (f + 1) * NF)
                    hc = mlp3(xp_s[32 * q:32 * q + 20, fs],
                              sb["w1a"][32 * q:32 * q + 20, :],
                              "b1a", "w1b", "b1b", "w1c", "b1c", 32, NF,
                              tp=(32 * q, 0))
                    nc.sync.dma_start(out=h[32 * q:32 * q + 32, fs], in_=hc[:])
            # px (x pooling) before xp slot is reused in phase 5
            px = spool.tile([128, cfg.JQ], FP, name="px")
            nc.vector.tensor_reduce(
                out=px[:], in_=xp_s[:].rearrange("p (j r) -> p j r", r=RJ),
                axis=AX.X, op=OP.add)
            # g = h * dinv (in place; tile "h" now holds g)
            nc.vector.tensor_tensor(out=h[:], in0=h[:], in1=dinv[:], op=OP.mult)
            g = h
            if dbg:
                nc.sync.dma_start(out=dbg_t["dbg_g"][:], in_=g[:])

            # ---- Phase 4: gather + scan + cum ----
            zrow = spool.tile([1, 8], FP, name="zrow")
            nc.vector.memset(zrow[:], 0.0)
            nc.sync.dma_start(out=cum[0:1, :], in_=zrow[:])
            gc0 = spool.tile([1, 8], FP, name="gc0")
            nc.vector.memset(gc0[:], 0.0)
            for c in range(NCH):
                msg = mpool.tile([128, KCH, 8], FP, name="msg", tag="msg")
                nc.sync.dma_start(
                    out=msg[:].rearrange("p k d -> p (k d)"),
                    in_=ge[c * cfg.CH:(c + 1) * cfg.CH, :].rearrange(
                        "(p k) d -> p (k d)", p=128))
                if dbg and c == NCH - 1:
                    nc.sync.dma_start(out=dbg_t["dbg_raw"][:], in_=msg[:])
                    last_msg = msg
                sc = mpool.tile([128, KCH, 8], FP, name="sc", tag="scan",
                                bufs=1)
                for chn in range(8):
                    nc.vector.tensor_tensor_scan(
                        out=sc[:, :, chn], data0=msg[:, :, chn],
                        data1=msg[:, :, chn], initial=0.0,
                        op0=OP.add, op1=OP.bypass)
                tots = wpool.tile([128, 8], FP, name="tots", tag="tots")
                nc.vector.tensor_copy(out=tots[:], in_=sc[:, KCH - 1, :])
                cps = psS.tile([128, 8], FP, name="cps", tag="S")
                nc.tensor.matmul(out=cps[:], lhsT=sb["triu"][:], rhs=tots[:],
                                 start=True, stop=False)
                nc.tensor.matmul(out=cps[:], lhsT=sb["ones1"][:],
                                 rhs=gc0[:], start=False, stop=True)
                ca = wpool.tile([128, 8], FP, name="ca", tag="ca")
                nc.vector.tensor_copy(out=ca[:], in_=cps[:])
                gps = psS.tile([1, 8], FP, name="gps", tag="S")
                nc.tensor.matmul(out=gps[:], lhsT=sb["onesc"][:], rhs=tots[:],
                                 start=True, stop=False)
                nc.tensor.matmul(out=gps[:], lhsT=sb["one11"][:], rhs=gc0[:],
                                 start=False, stop=True)
                nc.vector.tensor_copy(out=gc0[:], in_=gps[:])
                nc.vector.tensor_tensor(
                    out=sc[:], in0=sc[:],
                    in1=ca[:].unsqueeze(1).broadcast_to([128, KCH, 8]),
                    op=OP.add)
                nc.sync.dma_start(
                    out=cum[1 + c * cfg.CH: 1 + (c + 1) * cfg.CH, :].rearrange(
                        "(p k) d -> p (k d)", p=128),
                    in_=sc[:].rearrange("p k d -> p (k d)"))
                if dbg and c == NCH - 1:
                    nc.sync.dma_start(out=dbg_t["dbg_msg"][:], in_=sc[:])

            # ---- Phase 5: one boundary gather + on-chip shift -> aggr ----
            aggr = bigpool.tile([128, FQ], FP, name="aggr", tag="T2")
            TB2 = min(4, CCQ)
            CQ4E = CCQ * 4 + 4
            for q in range(4):
                qs = slice(q * CQ4E, (q + 1) * CQ4E)
                bs_q = mpool.tile([128, CQ4E], I32, name="bs_q", tag="colt")
                nc.sync.dma_start(out=bs_q[:], in_=io["bstart"][:, qs])
                stq = bigpool.tile([128, CQ4E, 8], FP, name="stq", tag="T4")
                for jj in range(CQ4E):
                    nc.gpsimd.indirect_dma_start(
                        out=stq[:, jj, :], out_offset=None, in_=cum[:],
                        in_offset=bass.IndirectOffsetOnAxis(
                            ap=bs_q[:, jj:jj + 1], axis=0))
                # ends[v] = starts[v+1]: i<3 is a free-dim shift; i=3 needs
                # the next partition's i=0 (p=127 -> next cc-block's p=0).
                st4 = stq[:].rearrange("p (cc i) d -> p cc i d", i=4)
                tmp = bigpool.tile([128, CCQ, 8], FP, name="tmp", tag="enq")
                nc.sync.dma_start(out=tmp[0:127, :, :],
                                  in_=st4[1:128, 0:CCQ, 0, :])
                nc.sync.dma_start(out=tmp[127:128, :, :],
                                  in_=st4[0:1, 1:CCQ + 1, 0, :])
                nc.vector.tensor_tensor(
                    out=st4[:, 0:CCQ, 0:3, :], in0=st4[:, 0:CCQ, 1:4, :],
                    in1=st4[:, 0:CCQ, 0:3, :], op=OP.subtract)
                nc.vector.tensor_tensor(
                    out=st4[:, 0:CCQ, 3, :], in0=tmp[:],
                    in1=st4[:, 0:CCQ, 3, :], op=OP.subtract)
                enq = stq
                for cb in range(CCQ // TB2):
                    pst = psC.tile([32, 128 * TB2], FP, name="psa", tag="C")
                    for t in range(TB2):
                        lc = cb * TB2 + t
                        nc.tensor.transpose(
                            out=pst[:, 128 * t:128 * t + 128],
                            in_=enq[:, 4 * lc:4 * lc + 4, :].rearrange(
                                "p i d -> p (i d)"),
                            identity=sb["id128"][:])
                    at = wpool.tile([32, 128 * TB2], FP, name="at",
                                    tag="t2k")
                    nc.vector.tensor_copy(out=at[:], in_=pst[:])
                    f0 = cb * TB2 * 128
                    nc.sync.dma_start(
                        out=aggr[32 * q:32 * q + 32, f0:f0 + 128 * TB2],
                        in_=at[:])

            nc.vector.tensor_tensor(out=aggr[:], in0=aggr[:], in1=g[:],
                                    op=OP.add)
            nc.vector.tensor_tensor(out=aggr[:], in0=aggr[:], in1=dinv[:],
                                    op=OP.mult)
            if dbg:
                nc.sync.dma_start(out=dbg_t["dbg_aggr"][:], in_=aggr[:])

            # ---- Phase 6: mlp2 -> x_node (tag T1, reuses g/h slot) ----
            xn = bigpool.tile([128, FQ], FP, name="xn", tag="T1")
            for q in range(4):
                for f in range(NFC):
                    fs = slice(f * NF, (f + 1) * NF)
                    xc = mlp3(aggr[32 * q:32 * q + 32, fs],
                              sb["w2a"][32 * q:32 * q + 32, :],
                              "b2a", "w2b", "b2b", "w2c", "b2c", 32, NF,
                              tp=(32 * q, 0))
                    nc.sync.dma_start(out=xn[32 * q:32 * q + 32, fs], in_=xc[:])

            # ---- Phase 7: pooling + mlp_dag + mlp_global ----
            pxn = spool.tile([128, cfg.JQ], FP, name="pxn")
            nc.vector.tensor_reduce(
                out=pxn[:], in_=xn[:].rearrange("p (j r) -> p j r", r=RJ),
                axis=AX.X, op=OP.add)
            sxp = psS.tile([5, cfg.JPC], FP, name="sxp", tag="S")
            sxnp = psS.tile([8, cfg.JPC], FP, name="sxnp", tag="S")
            for q in range(4):
                js = slice(q * cfg.JQ, (q + 1) * cfg.JQ)
                nc.tensor.matmul(out=sxp[:, js],
                                 lhsT=sb["selx"][:, 5 * q:5 * q + 5],
                                 rhs=px[:], start=True, stop=True)
                nc.tensor.matmul(out=sxnp[:, js],
                                 lhsT=sb["selxn"][:, 8 * q:8 * q + 8],
                                 rhs=pxn[:], start=True, stop=True)
            sx = spool.tile([5, cfg.JPC], FP, name="sx_s")
            sxn = spool.tile([8, cfg.JPC], FP, name="sxn_s")
            nc.vector.tensor_copy(out=sx[:], in_=sxp[:])
            nc.vector.tensor_copy(out=sxn[:], in_=sxnp[:])

            def mlp3s(rhs_list, ws, bs_, ms, name):
                cols_n = rhs_list[0][1].shape[-1]
                ps = psS.tile([ms[0], cols_n], FP, name=f"{name}p1", tag="S")
                for j, (wn, rap) in enumerate(rhs_list):
                    nc.tensor.matmul(out=ps[:], lhsT=sb[wn][:], rhs=rap,
                                     start=(j == 0),
                                     stop=(j == len(rhs_list) - 1))
                o1 = spool.tile([ms[0], cols_n], FP, name=f"{name}o1")
                nc.scalar.activation(out=o1[:], in_=ps[:], func=AF.Relu,
                                     bias=sb[bs_[0]][:, 0:1])
                ps2 = psS.tile([ms[1], cols_n], FP, name=f"{name}p2", tag="S")
                nc.tensor.matmul(out=ps2[:], lhsT=sb[ws[1]][:], rhs=o1[:],
                                 start=True, stop=True)
                o2 = spool.tile([ms[1], cols_n], FP, name=f"{name}o2")
                nc.scalar.activation(out=o2[:], in_=ps2[:], func=AF.Relu,
                                     bias=sb[bs_[1]][:, 0:1])
                ps3 = psS.tile([ms[2], cols_n], FP, name=f"{name}p3", tag="S")
                nc.tensor.matmul(out=ps3[:], lhsT=sb[ws[2]][:], rhs=o2[:],
                                 start=True, stop=True)
                o3 = spool.tile([ms[2], cols_n], FP, name=f"{name}o3")
                nc.scalar.activation(out=o3[:], in_=ps3[:], func=AF.Identity,
                                     bias=sb[bs_[2]][:, 0:1])
                return o3

            y = mlp3s([("wdx", sx[:]), ("wdxn", sxn[:])],
                      [None, "wd2", "wd3"], ["bd1", "bd2", "bd3"],
                      [32, 16, 8], "dag")
            if dbg:
                nc.sync.dma_start(out=dbg_t["dbg_y"][:], in_=y[:])
                nc.sync.dma_start(out=dbg_t["dbg_raw2"][:], in_=last_msg[:])
            yz = spool.tile([8, cfg.EVC], FP, name="yz")
            nc.vector.tensor_reduce(
                out=yz[:], in_=y[:].rearrange("p (e j) -> p e j", j=cfg.JPE),
                axis=AX.X, op=OP.add)
            z = mlp3s([("wg1", yz[:])],
                      [None, "wg2", "wg3"], ["bg1", "bg2", "bg3"],
                      [32, 16, 8], "glb")

            # ---- Phase 8: mlp_op -> ops_out ----
            JC = NF // RJ               # jobs per free-chunk
            EPN = cfg.OPJ * cfg.JPE // 4  # free elems per env per row
            for q in range(4):
                for f in range(NFC):
                    fs = slice(f * NF, (f + 1) * NF)
                    j0 = q * cfg.JQ + f * JC
                    e0 = (q * FQ + f * NF) // EPN
                    yr = wpool.tile([8, NF], FP, name="yr", tag="yr")
                    nc.vector.tensor_copy(
                        out=yr[:].rearrange("p (j r) -> p j r", r=RJ),
                        in_=y[:, j0:j0 + JC].unsqueeze(2).broadcast_to(
                            [8, JC, RJ]))
                    zr = wpool.tile([8, NF], FP, name="zr", tag="zr")
                    if EPN >= NF:
                        nc.vector.tensor_copy(
                            out=zr[:],
                            in_=z[:, e0:e0 + 1].broadcast_to([8, NF]))
                    else:
                        ec_n = NF // EPN
                        nc.vector.tensor_copy(
                            out=zr[:].rearrange("p (e r) -> p e r", r=EPN),
                            in_=z[:, e0:e0 + ec_n].unsqueeze(2).broadcast_to(
                                [8, ec_n, EPN]))
                    ps1 = psA.tile([128, NF], FP, name="po1", tag="A")
                    nc.tensor.matmul(out=ps1[:],
                                     lhsT=sb["wox"][32 * q:32 * q + 32, :],
                                     rhs=xn[32 * q:32 * q + 32, fs],
                                     start=True, stop=False,
                                     tile_position=(32 * q, 0))
                    nc.tensor.matmul(out=ps1[:], lhsT=sb["woy"][:], rhs=yr[:],
                                     start=False, stop=False)
                    nc.tensor.matmul(out=ps1[:], lhsT=sb["woz"][:], rhs=zr[:],
                                     start=False, stop=True)
                    a1 = wpool.tile([128, NF], FP, name="oa1", tag="a1")
                    nc.scalar.activation(out=a1[:], in_=ps1[:], func=AF.Relu,
                                         bias=sb["bo1"][:, 0:1])
                    ps2 = psB.tile([64, NF], FP, name="po2", tag="B")
                    nc.tensor.matmul(out=ps2[:], lhsT=sb["wo2"][:], rhs=a1[:],
                                     start=True, stop=True)
                    a2 = wpool.tile([64, NF], FP, name="oa2", tag="a2")
                    nc.scalar.activation(out=a2[:], in_=ps2[:], func=AF.Relu,
                                         bias=sb["bo2"][:, 0:1])
                    ps3 = psC.tile([4, NF], FP, name="po3", tag="C")
                    nc.tensor.matmul(out=ps3[:], lhsT=sb["wo3"][:], rhs=a2[:],
                                     start=True, stop=True)
                    oc = wpool.tile([4, NF], FP, name="oc", tag="t2k")
                    nc.scalar.activation(out=oc[:], in_=ps3[:],
                                         func=AF.Identity,
                                         bias=sb["bo3"][:, 0:1])
                    nc.sync.dma_start(out=ops_out[4 * q:4 * q + 4, fs],
                                      in_=oc[:])

            # ---- Phase 9: prlvl -> prl_out ----
            RW = cfg.WPAD // 4
            REW = cfg.WPAD * cfg.JPE // 4
            lim_s = bigpool.tile([4, cfg.MPRL], FP, name="lim_s", tag="T4")
            nc.sync.dma_start(out=lim_s[:], in_=io["limits"][:])
            yrp = bigpool.tile([8, cfg.MPRL], FP, name="yrp", tag="T3")
            nc.vector.tensor_copy(
                out=yrp[:].rearrange("p (j r) -> p j r", r=RW),
                in_=y[:].unsqueeze(2).broadcast_to([8, cfg.JPC, RW]))
            zrp = bigpool.tile([8, cfg.MPRL], FP, name="zrp", tag="enq")
            nc.vector.tensor_copy(
                out=zrp[:].rearrange("p (e r) -> p e r", r=REW),
                in_=z[:].unsqueeze(2).broadcast_to([8, cfg.EVC, REW]))
            f0 = 0
            while f0 < cfg.MPRL:
                nf = min(512, cfg.MPRL - f0)
                fs = slice(f0, f0 + nf)
                ps1 = psA.tile([128, nf], FP, name="pp1", tag="A")
                nc.tensor.matmul(out=ps1[:], lhsT=sb["wpl"][:],
                                 rhs=lim_s[:, fs], start=True, stop=False)
                nc.tensor.matmul(out=ps1[:], lhsT=sb["wpy"][:], rhs=yrp[:, fs],
                                 start=False, stop=False)
                nc.tensor.matmul(out=ps1[:], lhsT=sb["wpz"][:], rhs=zrp[:, fs],
                                 start=False, stop=True)
                a1 = wpool.tile([128, nf], FP, name="pa1", tag="a1")
                nc.scalar.activation(out=a1[:], in_=ps1[:], func=AF.Relu,
                                     bias=sb["bp1"][:, 0:1])
                ps2 = psB.tile([64, nf], FP, name="pp2", tag="B")
                nc.tensor.matmul(out=ps2[:], lhsT=sb["wp2"][:], rhs=a1[:],
                                 start=True, stop=True)
                a2 = wpool.tile([64, nf], FP, name="pa2", tag="a2")
                nc.scalar.activation(out=a2[:], in_=ps2[:], func=AF.Relu,
                                     bias=sb["bp2"][:, 0:1])
                ps3 = psC.tile([4, nf], FP, name="pp3", tag="C")
                nc.tensor.matmul(out=ps3[:], lhsT=sb["wp3"][:], rhs=a2[:],
                                 start=True, stop=True)
                oc = wpool.tile([4, nf], FP, name="poc", tag="t2k")
                nc.scalar.activation(out=oc[:], in_=ps3[:], func=AF.Identity,
                                     bias=sb["bp3"][:, 0:1])
                nc.sync.dma_start(out=prl_out[:, fs], in_=oc[:])
                f0 += nf

    nc.compile()
    return nc


# ---------------------------------------------------------------------------
# Host postprocessing
# ---------------------------------------------------------------------------

def host_post(cfg, results, batch, num_jobs_per_env):
    ops = []
    prl = []
    for c in range(cfg.NC):
        b = np.asarray(results[c]["ops_out"])          # [16, FQ]
        t = b.reshape(4, 4, cfg.FQ).transpose(0, 2, 1).reshape(cfg.NPC)
        ops.append(t)
        p = np.asarray(results[c]["prl_out"])          # [4, MPRL]
        rows = p.T.reshape(cfg.JPC * cfg.WPAD)
        prl.append(rows.reshape(cfg.JPC, cfg.WPAD)[:, :cfg.W1])
    op_scores = np.concatenate(ops).astype(np.float32)
    prlvl = np.concatenate(prl, axis=0).astype(np.float32)

    njpe = np.asarray(num_jobs_per_env)
    total_jobs = int(njpe.sum())
    batch = np.asarray(batch)
    nopj = np.bincount(batch, minlength=total_jobs).astype(np.int32)
    env_ids = np.repeat(np.arange(njpe.shape[0]), njpe)
    nope = np.zeros(njpe.shape[0], dtype=np.int32)
    np.add.at(nope, env_ids, nopj)
    job_indptr = np.concatenate([np.zeros(1, np.int32),
                                 np.cumsum(njpe).astype(np.int32)])
    return op_scores, prlvl, nope, job_indptr


# ---------------------------------------------------------------------------
# Entry points
# ---------------------------------------------------------------------------

_CACHE = {}


def get_nc(cfg, shared):
    if cfg not in _CACHE:
        _CACHE[cfg] = build(cfg, shared)
    return _CACHE[cfg]


def make_in_maps(cfg, shared, per_core):
    in_maps = []
    for c in range(cfg.NC):
        m = dict(shared)
        m.update(per_core[c])
        in_maps.append(m)
    return in_maps


def run(cfg, x, edge_index, batch, num_jobs_per_env, n_workers, params,
        trace=False):
    shared, per_core = host_prep(cfg, x, edge_index, params)
    nc = get_nc(cfg, shared)
    in_maps = make_in_maps(cfg, shared, per_core)
    res = run_bass_kernel_spmd(nc, in_maps, core_ids=list(range(cfg.NC)),
                               trace=trace)
    out = host_post(cfg, res.results, batch, num_jobs_per_env)
    return out, res


def kernel(x, edge_index, batch, num_jobs_per_env, n_workers, params):
    cfg = FULL
    assert int(n_workers) == cfg.NW
    out, _ = run(cfg, x, edge_index, batch, num_jobs_per_env, n_workers,
                 params)
    return out
